# revision 1
# baseline (speedup 1.0000x reference)
"""Causal self-attention (B=2, T=2048, C=1024, H=16, RoPE) on 8 TRN2 cores.

Sharding: data-parallel over B (2 groups of 4 cores) x tensor-parallel over
heads (4 heads per core). Each core computes q/k/v projections for its heads,
RoPE, causal attention, and its partial output projection; the host sums the
4 partial projections per batch and adds bp.

Layout choices (per core):
  - xT [C, T] resident in SBUF (contraction dim C on partitions).
  - q, k produced TRANSPOSED: qT/kT [256=4heads*64, T] via lhsT=W, rhs=xT.
    Head-dim pairs are pre-permuted (evens|odds) in the weights so RoPE
    needs no strided access; the pair-swap is a constant permutation
    matmul (J), combine on VectorE with f32 cos/sin.
  - v produced NON-transposed: [T, 256] via lhsT=xT, rhs=WvT.
  - scores computed transposed: ST[tk, tq] = k_rot @ q_rot^T per head, so
    softmax-exp is elementwise (ScalarE, scale=1/8 folded in), the causal
    mask is a fixed 128x128 triangle on diagonal blocks, fully-masked
    blocks are skipped, and P@V needs no transposes.
  - softmax denominators: VectorE accumulates exp-blocks, one ones-vector
    matmul reduces over partitions, reciprocal, then a constant broadcast
    matmul (EA) expands denominators back over partitions for the scale.
"""

import math

import numpy as np
import ml_dtypes

import concourse.bass as bass
import concourse.bacc as bacc
import concourse.mybir as mybir
from concourse.tile import TileContext
from concourse.bass_utils import run_bass_kernel_spmd

BF16 = mybir.dt.bfloat16
F32 = mybir.dt.float32
NPBF16 = ml_dtypes.bfloat16

N_CORES = 8
P = 128

_UNIFIED_ACT_SET = "natural_log_exp_and_others"


def _patch_act_tables():
    import concourse.hw_specs as _hw
    import concourse.bacc as _bacc
    if getattr(_bacc, "_act_tables_patched", False):
        return
    _orig = _hw.get_activation_tables

    def _gat(arch):
        tabs = _orig(arch)
        if _UNIFIED_ACT_SET in tabs:
            keep = tabs[_UNIFIED_ACT_SET]
            drop = {
                mybir.ActivationFunctionType.Exp,
                mybir.ActivationFunctionType.Copy,
            } & keep
            for name, fns in tabs.items():
                if name != _UNIFIED_ACT_SET:
                    for f in drop:
                        fns.discard(f)
        return tabs

    _bacc.get_activation_tables = _gat
    _bacc._act_tables_patched = True


def build_attention_kernel(nc, T=2048, C=1024, n_heads=4, hd=64):
    """Emit the per-core kernel. Returns nothing; tensors are declared on nc."""
    _patch_act_tables()
    HD = n_heads * hd            # 256: local head dims
    KC = C // P                  # 8: contraction chunks for projections
    NJC = HD // P                # 2: partition tiles of qT/kT (head pairs)
    TQB = 512                    # tq block for scores/PV
    NQB = T // TQB               # 4
    NKC = T // P                 # 16: tk chunks
    NTT = T // P                 # 16: t tiles for v
    scale = 1.0 / math.sqrt(hd)

    # ---- DRAM I/O ----
    xT = nc.declare_dram_parameter("xT", [C, T], BF16, isOutput=False)
    wqT = nc.declare_dram_parameter("wqT", [C, HD], BF16, isOutput=False)
    wkT = nc.declare_dram_parameter("wkT", [C, HD], BF16, isOutput=False)
    wvT = nc.declare_dram_parameter("wvT", [C, HD], BF16, isOutput=False)
    wpT = nc.declare_dram_parameter("wpT", [HD, C], BF16, isOutput=False)
    cosq = nc.declare_dram_parameter("cosq", [P, T], F32, isOutput=False)
    sinsq = nc.declare_dram_parameter("sinsq", [P, T], F32, isOutput=False)
    jmat = nc.declare_dram_parameter("jmat", [P, P], BF16, isOutput=False)
    tri = nc.declare_dram_parameter("tri", [P, P], BF16, isOutput=False)
    ea = nc.declare_dram_parameter("ea", [P, P], BF16, isOutput=False)
    onesc = nc.declare_dram_parameter("onesc", [P, 1], BF16, isOutput=False)
    bqT = nc.declare_dram_parameter("bqT", [P, NJC], F32, isOutput=False)
    bkT = nc.declare_dram_parameter("bkT", [P, NJC], F32, isOutput=False)
    bvb = nc.declare_dram_parameter("bvb", [P, HD], F32, isOutput=False)
    z = nc.declare_dram_parameter("z", [T, C], F32, isOutput=True)

    with TileContext(nc) as tc:
        import contextlib

        with contextlib.ExitStack() as ctx:
            # ---- persistent SBUF pools ----
            pc = ctx.enter_context(tc.tile_pool(name="const", bufs=1))
            px = ctx.enter_context(tc.tile_pool(name="x", bufs=1))
            pw = ctx.enter_context(tc.tile_pool(name="w", bufs=1))
            pqk = ctx.enter_context(tc.tile_pool(name="qk", bufs=1))
            pv = ctx.enter_context(tc.tile_pool(name="v", bufs=1))
            py = ctx.enter_context(tc.tile_pool(name="y", bufs=1))
            # transient pools
            pf32 = ctx.enter_context(tc.tile_pool(name="f32tmp", bufs=2))
            prt = ctx.enter_context(tc.tile_pool(name="ropetmp", bufs=2))
            pexp = ctx.enter_context(tc.tile_pool(name="exp", bufs=6))
            pacc = ctx.enter_context(tc.tile_pool(name="acc", bufs=3))
            prcp = ctx.enter_context(tc.tile_pool(name="rcp", bufs=3))
            # PSUM pools
            pmm = ctx.enter_context(
                tc.tile_pool(name="mm", bufs=2, space="PSUM"))
            pyt = ctx.enter_context(
                tc.tile_pool(name="yt", bufs=2, space="PSUM"))

            # ---- constant + weight loads ----
            t_j = pc.tile([P, P], BF16, tag="j")
            nc.gpsimd.dma_start(t_j[:], jmat[:])
            t_tri = pc.tile([P, P], BF16, tag="tri")
            nc.gpsimd.dma_start(t_tri[:], tri[:])
            t_ea = pc.tile([P, P], BF16, tag="ea")
            nc.gpsimd.dma_start(t_ea[:], ea[:])
            t_ones = pc.tile([P, 1], BF16, tag="ones")
            nc.gpsimd.dma_start(t_ones[:], onesc[:])
            t_bq = pc.tile([P, NJC], F32, tag="bq")
            nc.gpsimd.dma_start(t_bq[:], bqT[:])
            t_bk = pc.tile([P, NJC], F32, tag="bk")
            nc.gpsimd.dma_start(t_bk[:], bkT[:])
            t_bv = pc.tile([P, HD], F32, tag="bv")
            nc.gpsimd.dma_start(t_bv[:], bvb[:])
            t_cos = pc.tile([P, T], F32, tag="cos")
            nc.gpsimd.dma_start(t_cos[:], cosq[:])
            t_sin = pc.tile([P, T], F32, tag="sin")
            nc.gpsimd.dma_start(t_sin[:], sinsq[:])
            # persistent staging tile for softmax denominators (rows 0/64
            # carry data; the rest must be finite zeros for the EA matmul)
            t_scp = pc.tile([P, 512], BF16, tag="scp")
            nc.vector.memset(t_scp[:], 0.0)

            t_wq = []
            t_wk = []
            t_wv = []
            for k in range(KC):
                wq_t = pw.tile([P, HD], BF16, tag=f"wq{k}")
                nc.gpsimd.dma_start(wq_t[:], wqT[k * P:(k + 1) * P, :])
                t_wq.append(wq_t)
                wk_t = pw.tile([P, HD], BF16, tag=f"wk{k}")
                nc.gpsimd.dma_start(wk_t[:], wkT[k * P:(k + 1) * P, :])
                t_wk.append(wk_t)
                wv_t = pw.tile([P, HD], BF16, tag=f"wv{k}")
                nc.gpsimd.dma_start(wv_t[:], wvT[k * P:(k + 1) * P, :])
                t_wv.append(wv_t)
            t_wp = []
            for jc in range(NJC):
                wp_t = pw.tile([P, C], BF16, tag=f"wp{jc}")
                nc.gpsimd.dma_start(wp_t[:], wpT[jc * P:(jc + 1) * P, :])
                t_wp.append(wp_t)

            t_x = []
            for k in range(KC):
                x_t = px.tile([P, T], BF16, tag=f"x{k}")
                nc.gpsimd.dma_start(x_t[:], xT[k * P:(k + 1) * P, :])
                t_x.append(x_t)

            # ---- v projection: v[t, dv] in 16 tiles [128, HD] ----
            t_v = []
            for tt in range(NTT):
                vps = pmm.tile([P, TQB], F32, tag="mm")
                for k in range(KC):
                    nc.tensor.matmul(
                        vps[:, 0:HD],
                        lhsT=t_x[k][:, tt * P:(tt + 1) * P],
                        rhs=t_wv[k][:],
                        start=(k == 0),
                        stop=(k == KC - 1),
                    )
                # v layout [128, 4*65]: head i at cols [i*65, i*65+64),
                # a ones column at i*65+64 (PV with it computes the softmax
                # denominator for free as an extra output row)
                v_t = pv.tile([P, n_heads * (hd + 1)], BF16, tag=f"v{tt}")
                v3 = v_t[:].rearrange("p (h c) -> p h c", h=n_heads)
                nc.vector.tensor_add(
                    v3[:, :, 0:hd],
                    vps[:, 0:HD].rearrange("p (h c) -> p h c", h=n_heads),
                    t_bv[:].rearrange("p (h c) -> p h c", h=n_heads),
                )
                nc.gpsimd.memset(v3[:, :, hd:hd + 1], 1.0)
                t_v.append(v_t)

            # ---- q/k projections (transposed) + RoPE ----
            # qrot/krot: NJC tiles [128, T] bf16
            t_qrot = [pqk.tile([P, T], BF16, tag=f"qr{jc}", name=f"qrot{jc}")
                      for jc in range(NJC)]
            t_krot = [pqk.tile([P, T], BF16, tag=f"kr{jc}", name=f"krot{jc}")
                      for jc in range(NJC)]

            for jc in range(NJC):
                for (wchunks, bias, dst) in (
                    (t_wq, t_bq, t_qrot[jc]),
                    (t_wk, t_bk, t_krot[jc]),
                ):
                    raw = pf32.tile([P, T], BF16, tag="qkraw")
                    for tb in range(T // TQB):
                        qps = pmm.tile([P, TQB], F32, tag="mm")
                        for k in range(KC):
                            nc.tensor.matmul(
                                qps[:],
                                lhsT=wchunks[k][:, jc * P:(jc + 1) * P],
                                rhs=t_x[k][:, tb * TQB:(tb + 1) * TQB],
                                start=(k == 0),
                                stop=(k == KC - 1),
                            )
                        # evacuate + bias (bias cols are per-partition scalars)
                        nc.vector.tensor_scalar_add(
                            raw[:, tb * TQB:(tb + 1) * TQB],
                            qps[:],
                            bias[:, jc:jc + 1],
                        )
                    # RoPE: rot = cos*raw + sins*(J@raw)
                    RW = min(1024, T)
                    for half in range(T // RW):
                        sl = slice(half * RW, (half + 1) * RW)
                        jps = pmm.tile([P, RW], F32, tag="mm")
                        for qtr in range(RW // TQB):
                            nc.tensor.matmul(
                                jps[:, qtr * TQB:(qtr + 1) * TQB],
                                lhsT=t_j[:],
                                rhs=raw[:, sl][:, qtr * TQB:(qtr + 1) * TQB],
                            )
                        tmp1 = prt.tile([P, RW], F32, tag="rope1")
                        nc.vector.tensor_mul(tmp1[:], raw[:, sl], t_cos[:, sl])
                        tmp2 = prt.tile([P, RW], F32, tag="rope2")
                        nc.vector.tensor_mul(tmp2[:], jps[:], t_sin[:, sl])
                        nc.vector.tensor_add(dst[:, sl], tmp1[:], tmp2[:])

            # ---- y_norm accumulators ----
            t_yn = [py.tile([P, T], BF16, tag=f"yn{jc}", name=f"yn{jc}")
                    for jc in range(NJC)]

            # ---- attention (qb outer so proj/z-DMA interleave) ----
            for qb in range(NQB):
                for hp in range(NJC):
                    n_kc = min(NKC, (qb + 1) * (TQB // P))
                    i0, i1 = hp * 2, hp * 2 + 1
                    # h-even: psum rows 0-63 = y, row 64 = denominators
                    yt_a = pyt.tile([P, TQB], F32, tag="yta")
                    # h-odd: psum rows 64-127 = y, row 0 = denominators
                    yt_b = pyt.tile([P, TQB], F32, tag="ytb")
                    acc = pacc.tile([P, TQB], BF16, tag="acc")
                    for kc in range(n_kc):
                        # scores for both heads of the pair -> one 2-bank tile
                        sc = pmm.tile([P, 2 * TQB], F32, tag="mm")
                        for hl in range(2):
                            nc.tensor.matmul(
                                sc[:, hl * TQB:(hl + 1) * TQB],
                                lhsT=t_krot[hp][
                                    hl * hd:(hl + 1) * hd,
                                    kc * P:(kc + 1) * P],
                                rhs=t_qrot[hp][
                                    hl * hd:(hl + 1) * hd,
                                    qb * TQB:(qb + 1) * TQB],
                            )
                        # exp with 1/sqrt(hd) folded in; diag-trim left cols
                        s0 = max(0, kc * P - qb * TQB)
                        ex = pexp.tile([P, 2 * TQB], BF16, tag="exp")
                        sc3 = sc[:].rearrange("p (h w) -> p h w", h=2)
                        ex3 = ex[:].rearrange("p (h w) -> p h w", h=2)
                        if s0 > 0:
                            nc.gpsimd.memset(ex3[:, :, 0:s0], 0.0)
                        nc.scalar.activation(
                            ex3[:, :, s0:TQB],
                            sc3[:, :, s0:TQB],
                            mybir.ActivationFunctionType.Exp,
                            scale=scale,
                        )
                        # diagonal 128-wide triangle mask (tk<=tq kept)
                        if kc * P >= qb * TQB:
                            tri3 = bass.AP(
                                t_tri.tensor, t_tri[:].offset,
                                [t_tri[:].ap[0], [0, 2], t_tri[:].ap[1]],
                            )
                            nc.gpsimd.tensor_mul(
                                ex3[:, :, s0:s0 + P],
                                ex3[:, :, s0:s0 + P],
                                tri3,
                            )
                        # h-odd denominator accumulate on VectorE
                        if kc == 0:
                            nc.vector.tensor_copy(acc[:], ex[:, TQB:2 * TQB])
                        else:
                            nc.vector.tensor_add(
                                acc[:], acc[:], ex[:, TQB:2 * TQB])
                        # P @ V (h-even carries the ones column -> row 64)
                        nc.tensor.matmul(
                            yt_a[0:hd + 1, :],
                            lhsT=t_v[kc][:, i0 * (hd + 1):i0 * (hd + 1) + hd + 1],
                            rhs=ex[:, 0:TQB],
                            start=(kc == 0),
                            stop=(kc == n_kc - 1),
                            skip_group_check=True,
                        )
                        nc.tensor.matmul(
                            yt_b[hd:2 * hd, :],
                            lhsT=t_v[kc][:, i1 * (hd + 1):i1 * (hd + 1) + hd],
                            rhs=ex[:, TQB:2 * TQB],
                            start=(kc == 0),
                            stop=(kc == n_kc - 1),
                            skip_group_check=True,
                        )
                    # h-odd denominators: reduce over partitions into yt_b row 0
                    nc.tensor.matmul(
                        yt_b[0:1, :], lhsT=t_ones[:], rhs=acc[:],
                        skip_group_check=True,
                    )
                    # stage both denominator rows, broadcast via EA matmul,
                    # then 1/s = exp(-ln(s)) on ScalarE
                    with nc.allow_low_precision(reason="bf16 softmax denom"):
                        nc.vector.tensor_copy(
                            t_scp[0:1, :], yt_a[hd:hd + 1, :])
                        nc.vector.tensor_copy(
                            t_scp[hd:hd + 1, :], yt_b[0:1, :])
                    bc = pmm.tile([P, 2 * TQB], F32, tag="mm", name="bc")
                    nc.tensor.matmul(
                        bc[:, 0:TQB], lhsT=t_ea[:], rhs=t_scp[:])
                    rcpb = prcp.tile([P, TQB], F32, tag="rcpb")
                    nc.scalar.activation(
                        rcpb[:], bc[:, 0:TQB],
                        mybir.ActivationFunctionType.Ln)
                    nc.scalar.activation(
                        rcpb[:], rcpb[:], mybir.ActivationFunctionType.Exp,
                        scale=-1.0)
                    nc.vector.tensor_mul(
                        t_yn[hp][0:hd, qb * TQB:(qb + 1) * TQB],
                        yt_a[0:hd, :], rcpb[0:hd, :])
                    nc.vector.tensor_mul(
                        t_yn[hp][hd:2 * hd, qb * TQB:(qb + 1) * TQB],
                        yt_b[hd:2 * hd, :], rcpb[hd:2 * hd, :])

                # ---- output projection for this qb ----
                for m in range(TQB // P):
                    tt = qb * (TQB // P) + m
                    for co in range(C // TQB):
                        zps = pmm.tile([P, 2 * TQB], F32, tag="mm")
                        for jc in range(NJC):
                            nc.tensor.matmul(
                                zps[:, 0:TQB],
                                lhsT=t_yn[jc][:, tt * P:(tt + 1) * P],
                                rhs=t_wp[jc][:, co * TQB:(co + 1) * TQB],
                                start=(jc == 0),
                                stop=(jc == NJC - 1),
                            )
                        zev = pf32.tile([P, TQB], F32, tag="zev", bufs=3)
                        nc.scalar.activation(
                            zev[:], zps[:, 0:TQB],
                            mybir.ActivationFunctionType.Copy)
                        nc.gpsimd.dma_start(
                            z[tt * P:(tt + 1) * P, co * TQB:(co + 1) * TQB],
                            zev[:],
                        )

_ROPE_PERM = np.concatenate([np.arange(0, 64, 2), np.arange(1, 64, 2)])


def _host_inputs(x_b, Wq, bq, Wk, bk, Wv, bv, Wp, heads, T, C, hd):
    """Build the per-core DRAM input dict (numpy)."""
    HD = len(heads) * hd
    rows = np.concatenate([h * hd + _ROPE_PERM for h in heads])
    rows_nop = np.concatenate([np.arange(h * hd, (h + 1) * hd) for h in heads])

    xT = np.ascontiguousarray(x_b.T).astype(NPBF16)
    wqT = np.ascontiguousarray(Wq[rows].T).astype(NPBF16)
    wkT = np.ascontiguousarray(Wk[rows].T).astype(NPBF16)
    wvT = np.ascontiguousarray(Wv[rows_nop].T).astype(NPBF16)
    wpT = np.ascontiguousarray(Wp[:, rows_nop].T).astype(NPBF16)

    j = np.arange(hd // 2, dtype=np.float64)
    inv_freq = 1.0 / (10000.0 ** (2.0 * j / hd))
    t = np.arange(T, dtype=np.float64)
    ang = t[:, None] * inv_freq[None, :]          # [T, 32]
    cos = np.cos(ang)
    sin = np.sin(ang)
    r = np.arange(P)
    cosq = cos[:, r % (hd // 2)].T.astype(np.float32)
    sgn = np.where((r % hd) < hd // 2, -1.0, 1.0)
    sinsq = (sin[:, r % (hd // 2)] * sgn[None, :]).T.astype(np.float32)
    cosq = np.ascontiguousarray(cosq)
    sinsq = np.ascontiguousarray(sinsq)

    pair = np.where((r % hd) < hd // 2, r + hd // 2, r - hd // 2)
    jmat = np.zeros((P, P), np.float32)
    jmat[pair, r] = 1.0
    tri = (np.arange(P)[None, :] >= np.arange(P)[:, None]).astype(np.float32)
    ea = np.zeros((P, P), np.float32)
    ea[(r // hd) * hd, r] = 1.0

    bqTh = bq[rows].reshape(HD // P, P).T.astype(np.float32)
    bkTh = bk[rows].reshape(HD // P, P).T.astype(np.float32)
    bvb = np.tile(bv[rows_nop][None, :], (P, 1)).astype(np.float32)

    return {
        "xT": xT, "wqT": wqT, "wkT": wkT, "wvT": wvT, "wpT": wpT,
        "cosq": cosq, "sinsq": sinsq,
        "jmat": jmat.astype(NPBF16), "tri": tri.astype(NPBF16),
        "ea": ea.astype(NPBF16),
        "onesc": np.ones((P, 1), NPBF16),
        "bqT": np.ascontiguousarray(bqTh),
        "bkT": np.ascontiguousarray(bkTh),
        "bvb": bvb,
    }


def make_core_inputs(x, Wq, bq, Wk, bk, Wv, bv, Wp, T=2048, C=1024, hd=64,
                     heads_per_core=4):
    in_maps = []
    for c in range(N_CORES):
        b = c // 4
        g = c % 4
        heads = list(range(g * heads_per_core, (g + 1) * heads_per_core))
        in_maps.append(_host_inputs(
            np.asarray(x[b]), Wq, bq, Wk, bk, Wv, bv, Wp, heads, T, C, hd))
    return in_maps


def kernel(x, Wq, bq, Wk, bk, Wv, bv, Wp, bp):
    x = np.asarray(x, np.float32)
    Wq = np.asarray(Wq, np.float32)
    bq = np.asarray(bq, np.float32)
    Wk = np.asarray(Wk, np.float32)
    bk = np.asarray(bk, np.float32)
    Wv = np.asarray(Wv, np.float32)
    bv = np.asarray(bv, np.float32)
    Wp = np.asarray(Wp, np.float32)
    bp = np.asarray(bp, np.float32)
    B, T, C = x.shape

    _patch_act_tables()
    nc = bacc.Bacc("TRN2", target_bir_lowering=False, debug=False,
                   num_devices=N_CORES)
    build_attention_kernel(nc, T=T, C=C)
    nc.compile()

    in_maps = make_core_inputs(x, Wq, bq, Wk, bk, Wv, bv, Wp, T=T, C=C)
    res = run_bass_kernel_spmd(nc, in_maps, list(range(N_CORES)))

    out = np.zeros((B, T, C), np.float32)
    for c in range(N_CORES):
        out[c // 4] += res.results[c]["z"]
    out += bp[None, None, :]
    return out


if __name__ == "__main__":
    import reference

    inputs = reference.setup_inputs()
    expected = np.asarray(reference.reference(**inputs))
    actual = kernel(**{k: np.asarray(v) for k, v in inputs.items()})
    err = np.abs(actual - expected).max() / np.abs(expected).max()
    print("Relative error:", err)



# revision 6
# speedup vs baseline: 1.2418x; 1.2418x over previous
"""Causal self-attention (B=2, T=2048, C=1024, H=16, RoPE) on 8 TRN2 cores.

Sharding: data-parallel over B (2 groups of 4 cores) x tensor-parallel over
heads (4 heads per core). Each core computes q/k/v projections for its heads,
RoPE, causal attention, and its partial output projection; the host sums the
4 partial projections per batch and adds bp.

Layout choices (per core):
  - xT [C, T] resident in SBUF (contraction dim C on partitions), loaded with
    ONE batched DMA; weights/consts likewise batched (6 DMAs total, spread
    across engine queues so issue cost doesn't serialize).
  - q, k produced TRANSPOSED: qT/kT [256=4heads*64, T] via lhsT=W, rhs=xT.
    Head-dim pairs are pre-permuted (evens|odds) in the weights so RoPE
    needs no strided access; the pair-swap is a constant permutation
    matmul (J), combine on VectorE with f32 cos/sin.
  - v produced NON-transposed: [T, 256] via lhsT=xT, rhs=WvT.
  - scores computed transposed: ST[tk, tq] = k_rot @ q_rot^T per head; the
    two heads of a pair run CONCURRENTLY on the PE array (row-tiled, K=64
    each at row groups 0/64). softmax-exp is elementwise on ScalarE with
    scale=1/8 folded in; causal masking is a fixed 128x128 triangle on
    diagonal blocks; fully-masked blocks are skipped.
  - P@V also paired: M=64 per head (col groups 0/64) into one PSUM tile.
  - The attention loop is software-pipelined: scores(kc+1) is emitted before
    PV(kc) so the PE never stalls behind the exp of the current block.
  - softmax denominators: VectorE accumulates exp blocks (both heads), two
    1-row ones-matmuls reduce over partitions, EA-matmul broadcasts back,
    reciprocal on VectorE; y-normalization on VectorE.
  - output projection per qb; PSUM evacuated by GpSimd into a staging tile,
    one batched DMA per qb writes z.
"""

import math

import numpy as np
import ml_dtypes

import concourse.bass as bass
import concourse.bacc as bacc
import concourse.mybir as mybir
from concourse.tile import TileContext
from concourse.bass_utils import run_bass_kernel_spmd

BF16 = mybir.dt.bfloat16
F32 = mybir.dt.float32
NPBF16 = ml_dtypes.bfloat16

N_CORES = 8
P = 128


def build_attention_kernel(nc, T=2048, C=1024, n_heads=4, hd=64):
    """Emit the per-core kernel. Returns nothing; tensors are declared on nc."""
    HD = n_heads * hd            # 256: local head dims
    KC = C // P                  # 8: contraction chunks for projections
    NJC = HD // P                # 2: partition tiles of qT/kT (head pairs)
    TQB = 512                    # tq block for scores/PV
    NQB = T // TQB               # 4
    NTT = T // P                 # 16: t tiles for v
    scale = 1.0 / math.sqrt(hd)

    # ---- DRAM I/O ----
    xT = nc.declare_dram_parameter("xT", [C, T], BF16, isOutput=False)
    wqkv = nc.declare_dram_parameter("wqkv", [C, 3 * HD], BF16, isOutput=False)
    wpT = nc.declare_dram_parameter("wpT", [HD, C], BF16, isOutput=False)
    # f32 consts: bq(NJC) | bk(NJC) | bv(HD) | cos(T) | sin(T)
    NFC = 2 * NJC + HD + 2 * T
    fcst = nc.declare_dram_parameter("fcst", [P, NFC], F32, isOutput=False)
    # bf16 consts: j(P) | tri(P) | ea(P) | ones(2)
    NBC = 3 * P + 2
    bcst = nc.declare_dram_parameter("bcst", [P, NBC], BF16, isOutput=False)
    z = nc.declare_dram_parameter("z", [T, C], F32, isOutput=True)

    with TileContext(nc) as tc:
        import contextlib

        with contextlib.ExitStack() as ctx:
            # ---- persistent SBUF pools ----
            pc = ctx.enter_context(tc.tile_pool(name="const", bufs=1))
            px = ctx.enter_context(tc.tile_pool(name="x", bufs=1))
            pw = ctx.enter_context(tc.tile_pool(name="w", bufs=1))
            pqk = ctx.enter_context(tc.tile_pool(name="qk", bufs=1))
            pv = ctx.enter_context(tc.tile_pool(name="v", bufs=1))
            py = ctx.enter_context(tc.tile_pool(name="y", bufs=1))
            # transient pools
            praw = ctx.enter_context(tc.tile_pool(name="raw", bufs=2))
            prt = ctx.enter_context(tc.tile_pool(name="ropetmp", bufs=4))
            pexp = ctx.enter_context(tc.tile_pool(name="exp", bufs=6))
            pacc = ctx.enter_context(tc.tile_pool(name="acc", bufs=2))
            prcp = ctx.enter_context(tc.tile_pool(name="rcp", bufs=2))
            pzs = ctx.enter_context(tc.tile_pool(name="zstage", bufs=2))
            # PSUM pools (4 + 2 + 2 = 8 banks)
            psc = ctx.enter_context(
                tc.tile_pool(name="sc", bufs=2, space="PSUM"))
            pyt = ctx.enter_context(
                tc.tile_pool(name="yt", bufs=2, space="PSUM"))
            pms = ctx.enter_context(
                tc.tile_pool(name="ms", bufs=2, space="PSUM"))

            # ---- batched input DMAs, spread across engine queues ----
            t_x = px.tile([P, KC * T], BF16, tag="x")
            nc.sync.dma_start(
                t_x[:].rearrange("p (k t) -> p k t", k=KC),
                xT[:].rearrange("(k p) t -> p k t", k=KC),
            )
            t_w = pw.tile([P, KC * 3 * HD], BF16, tag="w")
            nc.scalar.dma_start(
                t_w[:].rearrange("p (k n) -> p k n", k=KC),
                wqkv[:].rearrange("(k p) n -> p k n", k=KC),
            )
            t_fc = pc.tile([P, NFC], F32, tag="fc")
            nc.scalar.dma_start(t_fc[:], fcst[:])
            t_bc = pc.tile([P, NBC], BF16, tag="bc")
            nc.gpsimd.dma_start(t_bc[:], bcst[:])
            t_wp = pw.tile([P, NJC * C], BF16, tag="wp")
            nc.gpsimd.dma_start(
                t_wp[:].rearrange("p (j n) -> p j n", j=NJC),
                wpT[:].rearrange("(j p) n -> p j n", j=NJC),
            )

            # const views
            t_j = t_bc[:, 0:P]
            t_tri = t_bc[:, P:2 * P]
            t_ea = t_bc[:, 2 * P:3 * P]
            t_ones = t_bc[:, 3 * P:3 * P + 1]
            bq = t_fc[:, 0:NJC]
            bk = t_fc[:, NJC:2 * NJC]
            bv = t_fc[:, 2 * NJC:2 * NJC + HD]
            cos = t_fc[:, 2 * NJC + HD:2 * NJC + HD + T]
            sin = t_fc[:, 2 * NJC + HD + T:NFC]

            # persistent staging tile for softmax denominators (rows 0/64
            # carry data; the rest must be finite zeros for the EA matmul)
            t_scp = pc.tile([P, TQB], BF16, tag="scp")
            nc.vector.memset(t_scp[:], 0.0)

            # tri view broadcast over the 2-head dim
            def tri3():
                ap = t_tri
                return bass.AP(
                    ap.tensor, ap.offset,
                    [ap.ap[0], [0, 2], ap.ap[1]],
                )

            # ---- q/k projections (transposed) + RoPE ----
            t_qrot = [pqk.tile([P, T], BF16, tag=f"qr{jc}", name=f"qrot{jc}")
                      for jc in range(NJC)]
            t_krot = [pqk.tile([P, T], BF16, tag=f"kr{jc}", name=f"krot{jc}")
                      for jc in range(NJC)]

            for jc in range(NJC):
                for (woff, bias, dst) in (
                    (0, bq, t_qrot[jc]),
                    (HD, bk, t_krot[jc]),
                ):
                    raw = praw.tile([P, T], BF16, tag="qkraw")
                    for tb in range(T // TQB):
                        qps = psc.tile([P, 2 * TQB], F32, tag="sc")
                        for k in range(KC):
                            nc.tensor.matmul(
                                qps[:, 0:TQB],
                                lhsT=t_w[:, k * 3 * HD + woff + jc * P:
                                         k * 3 * HD + woff + (jc + 1) * P],
                                rhs=t_x[:, k * T + tb * TQB:
                                        k * T + (tb + 1) * TQB],
                                start=(k == 0),
                                stop=(k == KC - 1),
                            )
                        # evacuate + bias (bias cols are per-partition scalars)
                        nc.vector.tensor_scalar_add(
                            raw[:, tb * TQB:(tb + 1) * TQB],
                            qps[:, 0:TQB],
                            bias[:, jc:jc + 1],
                        )
                    # RoPE: rot = cos*raw + sin*(J@raw)
                    RW = 1024
                    for half in range(T // RW):
                        sl = slice(half * RW, (half + 1) * RW)
                        jps = psc.tile([P, RW], F32, tag="sc")
                        for qtr in range(RW // TQB):
                            nc.tensor.matmul(
                                jps[:, qtr * TQB:(qtr + 1) * TQB],
                                lhsT=t_j,
                                rhs=raw[:, sl][:, qtr * TQB:(qtr + 1) * TQB],
                            )
                        tmp1 = prt.tile([P, RW], F32, tag="rope1")
                        nc.vector.tensor_mul(tmp1[:], raw[:, sl], cos[:, sl])
                        tmp2 = prt.tile([P, RW], F32, tag="rope2")
                        nc.vector.tensor_mul(tmp2[:], jps[:], sin[:, sl])
                        nc.vector.tensor_add(dst[:, sl], tmp1[:], tmp2[:])

            # ---- y_norm accumulators ----
            t_yn = [py.tile([P, T], BF16, tag=f"yn{jc}", name=f"yn{jc}")
                    for jc in range(NJC)]

            # ---- v projection emitter (chunks interleave with attention) ----
            t_v = [None] * NTT

            def emit_vproj(tt0, tt1):
                for tt in range(tt0, tt1):
                    vps = pyt.tile([P, TQB], F32, tag="yt")
                    for k in range(KC):
                        nc.tensor.matmul(
                            vps[:, 0:HD],
                            lhsT=t_x[:, k * T + tt * P:k * T + (tt + 1) * P],
                            rhs=t_w[:, k * 3 * HD + 2 * HD:(k + 1) * 3 * HD],
                            start=(k == 0),
                            stop=(k == KC - 1),
                        )
                    v_t = pv.tile([P, HD], BF16, tag=f"v{tt}")
                    nc.vector.tensor_add(v_t[:], vps[:, 0:HD], bv)
                    t_v[tt] = v_t

            # ---- attention (qb outer; vproj/zproj interleave) ----
            for qb in range(NQB):
                emit_vproj(qb * (TQB // P), (qb + 1) * (TQB // P))
                for hp in range(NJC):
                    n_kc = (qb + 1) * (TQB // P)
                    # software-pipelined: scores(kc+1) emitted before PV(kc)
                    scs = [None] * n_kc

                    def emit_scores(kc):
                        sc = psc.tile([P, 2 * TQB], F32, tag="sc")
                        for hl in range(2):
                            nc.tensor.matmul(
                                sc[:, hl * TQB:(hl + 1) * TQB],
                                lhsT=t_krot[hp][
                                    hl * hd:(hl + 1) * hd,
                                    kc * P:(kc + 1) * P],
                                rhs=t_qrot[hp][
                                    hl * hd:(hl + 1) * hd,
                                    qb * TQB:(qb + 1) * TQB],
                            )
                        scs[kc] = sc

                    yt = pyt.tile([P, TQB], F32, tag="yt")
                    acc = pacc.tile([P, 2 * TQB], BF16, tag="acc")
                    emit_scores(0)
                    for kc in range(n_kc):
                        sc = scs[kc]
                        # exp with 1/sqrt(hd) folded in; diag-trim left cols
                        s0 = max(0, kc * P - qb * TQB)
                        ex = pexp.tile([P, 2 * TQB], BF16, tag="exp")
                        sc3 = sc[:].rearrange("p (h w) -> p h w", h=2)
                        ex3 = ex[:].rearrange("p (h w) -> p h w", h=2)
                        if s0 > 0:
                            nc.vector.memset(ex3[:, :, 0:s0], 0.0)
                        nc.scalar.activation(
                            ex3[:, :, s0:TQB],
                            sc3[:, :, s0:TQB],
                            mybir.ActivationFunctionType.Exp,
                            scale=scale,
                        )
                        if kc + 1 < n_kc:
                            emit_scores(kc + 1)
                        # diagonal 128-wide triangle mask (tk<=tq kept)
                        if kc * P >= qb * TQB:
                            nc.vector.tensor_mul(
                                ex3[:, :, s0:s0 + P],
                                ex3[:, :, s0:s0 + P],
                                tri3(),
                            )
                        # denominator accumulate on VectorE (both heads)
                        if kc == 0:
                            nc.vector.tensor_copy(acc[:], ex[:])
                        else:
                            nc.vector.tensor_add(acc[:], acc[:], ex[:])
                        # P @ V: both heads concurrent (col groups 0/64)
                        for hl in range(2):
                            nc.tensor.matmul(
                                yt[hl * hd:(hl + 1) * hd, :],
                                lhsT=t_v[kc][
                                    :, (2 * hp + hl) * hd:
                                       (2 * hp + hl + 1) * hd],
                                rhs=ex[:, hl * TQB:(hl + 1) * TQB],
                                start=(kc == 0),
                                stop=(kc == n_kc - 1),
                                skip_group_check=True,
                            )
                    # denominators: reduce acc over partitions into rows 0/64
                    dns = pms.tile([P, TQB], F32, tag="ms")
                    for hl in range(2):
                        nc.tensor.matmul(
                            dns[hl * hd:hl * hd + 1, :],
                            lhsT=t_ones,
                            rhs=acc[:, hl * TQB:(hl + 1) * TQB],
                            skip_group_check=True,
                        )
                    # stage rows, broadcast via EA matmul, reciprocal on DVE
                    with nc.allow_low_precision(reason="bf16 softmax denom"):
                        nc.vector.tensor_copy(
                            t_scp[0:1, :], dns[0:1, :])
                        nc.vector.tensor_copy(
                            t_scp[hd:hd + 1, :], dns[hd:hd + 1, :])
                    bc = pms.tile([P, TQB], F32, tag="ms")
                    nc.tensor.matmul(bc[:], lhsT=t_ea, rhs=t_scp[:])
                    rcpb = prcp.tile([P, TQB], F32, tag="rcpb")
                    nc.vector.reciprocal(rcpb[:], bc[:])
                    nc.vector.tensor_mul(
                        t_yn[hp][:, qb * TQB:(qb + 1) * TQB],
                        yt[:], rcpb[:])

                # ---- output projection for this qb ----
                zs = pzs.tile([P, (TQB // P) * C], F32, tag="zs")
                for m in range(TQB // P):
                    tt = qb * (TQB // P) + m
                    for co in range(C // TQB):
                        zps = pms.tile([P, TQB], F32, tag="ms")
                        for jc in range(NJC):
                            nc.tensor.matmul(
                                zps[:],
                                lhsT=t_yn[jc][:, tt * P:(tt + 1) * P],
                                rhs=t_wp[:, jc * C + co * TQB:
                                         jc * C + (co + 1) * TQB],
                                start=(jc == 0),
                                stop=(jc == NJC - 1),
                            )
                        nc.vector.tensor_copy(
                            zs[:, (m * (C // TQB) + co) * TQB:
                               (m * (C // TQB) + co + 1) * TQB],
                            zps[:])
                nc.gpsimd.dma_start(
                    z[qb * TQB:(qb + 1) * TQB, :].rearrange(
                        "(m p) (co c) -> p m co c", m=TQB // P, co=C // TQB),
                    zs[:].rearrange(
                        "p (m co c) -> p m co c", m=TQB // P, co=C // TQB),
                )


_ROPE_PERM = np.concatenate([np.arange(0, 64, 2), np.arange(1, 64, 2)])


def _host_inputs(x_b, Wq, bq, Wk, bk, Wv, bv, Wp, heads, T, C, hd):
    """Build the per-core DRAM input dict (numpy)."""
    HD = len(heads) * hd
    rows = np.concatenate([h * hd + _ROPE_PERM for h in heads])
    rows_nop = np.concatenate([np.arange(h * hd, (h + 1) * hd) for h in heads])

    xT = np.ascontiguousarray(x_b.T).astype(NPBF16)
    wqkv = np.concatenate(
        [Wq[rows].T, Wk[rows].T, Wv[rows_nop].T], axis=1).astype(NPBF16)
    wpT = np.ascontiguousarray(Wp[:, rows_nop].T).astype(NPBF16)

    j = np.arange(hd // 2, dtype=np.float64)
    inv_freq = 1.0 / (10000.0 ** (2.0 * j / hd))
    t = np.arange(T, dtype=np.float64)
    ang = t[:, None] * inv_freq[None, :]          # [T, 32]
    cosv = np.cos(ang)
    sinv = np.sin(ang)
    r = np.arange(P)
    cosq = cosv[:, r % (hd // 2)].T.astype(np.float32)
    sgn = np.where((r % hd) < hd // 2, -1.0, 1.0)
    sinsq = (sinv[:, r % (hd // 2)] * sgn[None, :]).T.astype(np.float32)

    pair = np.where((r % hd) < hd // 2, r + hd // 2, r - hd // 2)
    jmat = np.zeros((P, P), np.float32)
    jmat[pair, r] = 1.0
    tri = (np.arange(P)[None, :] >= np.arange(P)[:, None]).astype(np.float32)
    ea = np.zeros((P, P), np.float32)
    ea[(r // hd) * hd, r] = 1.0
    ones2 = np.ones((P, 2), np.float32)

    NJC = HD // P
    bqTh = bq[rows].reshape(NJC, P).T.astype(np.float32)
    bkTh = bk[rows].reshape(NJC, P).T.astype(np.float32)
    bvb = np.tile(bv[rows_nop][None, :], (P, 1)).astype(np.float32)

    fcst = np.concatenate(
        [bqTh, bkTh, bvb, cosq, sinsq], axis=1).astype(np.float32)
    bcst = np.concatenate([jmat, tri, ea, ones2], axis=1).astype(NPBF16)

    return {
        "xT": xT, "wqkv": wqkv, "wpT": wpT,
        "fcst": np.ascontiguousarray(fcst),
        "bcst": np.ascontiguousarray(bcst),
    }


def make_core_inputs(x, Wq, bq, Wk, bk, Wv, bv, Wp, T=2048, C=1024, hd=64,
                     heads_per_core=4):
    in_maps = []
    for c in range(N_CORES):
        b = c // 4
        g = c % 4
        heads = list(range(g * heads_per_core, (g + 1) * heads_per_core))
        in_maps.append(_host_inputs(
            np.asarray(x[b]), Wq, bq, Wk, bk, Wv, bv, Wp, heads, T, C, hd))
    return in_maps


def kernel(x, Wq, bq, Wk, bk, Wv, bv, Wp, bp):
    x = np.asarray(x, np.float32)
    Wq = np.asarray(Wq, np.float32)
    bq = np.asarray(bq, np.float32)
    Wk = np.asarray(Wk, np.float32)
    bk = np.asarray(bk, np.float32)
    Wv = np.asarray(Wv, np.float32)
    bv = np.asarray(bv, np.float32)
    Wp = np.asarray(Wp, np.float32)
    bp = np.asarray(bp, np.float32)
    B, T, C = x.shape

    nc = bacc.Bacc("TRN2", target_bir_lowering=False, debug=False,
                   num_devices=N_CORES)
    build_attention_kernel(nc, T=T, C=C)
    nc.compile()

    in_maps = make_core_inputs(x, Wq, bq, Wk, bk, Wv, bv, Wp, T=T, C=C)
    res = run_bass_kernel_spmd(nc, in_maps, list(range(N_CORES)))

    out = np.zeros((B, T, C), np.float32)
    for c in range(N_CORES):
        out[c // 4] += res.results[c]["z"]
    out += bp[None, None, :]
    return out


if __name__ == "__main__":
    import reference

    inputs = reference.setup_inputs()
    expected = np.asarray(reference.reference(**inputs))
    actual = kernel(**{k: np.asarray(v) for k, v in inputs.items()})
    err = np.abs(actual - expected).max() / np.abs(expected).max()
    print("Relative error:", err)


# revision 7
# speedup vs baseline: 1.3176x; 1.0611x over previous
"""Causal self-attention (B=2, T=2048, C=1024, H=16, RoPE) on 8 TRN2 cores.

Sharding: data-parallel over B (2 groups of 4 cores) x tensor-parallel over
heads (4 heads per core). Each core computes q/k/v projections for its heads,
RoPE, causal attention, and its partial output projection; the host sums the
4 partial projections per batch and adds bp.

Layout choices (per core):
  - xT [C, T] resident in SBUF (contraction dim C on partitions), loaded as 8
    per-chunk DMAs so the first projection matmul starts as soon as chunk 0
    lands; weights/consts batched per matrix and spread across engine queues.
  - q, k produced TRANSPOSED: qT/kT [256=4heads*64, T] via lhsT=W, rhs=xT.
    Head-dim pairs are pre-permuted (evens|odds) in the weights so RoPE
    needs no strided access; the pair-swap is a constant permutation
    matmul (J), combine on VectorE with bf16 cos/sin.
  - v produced NON-transposed: [T, 256] via lhsT=xT, rhs=WvT.
  - scores computed transposed: ST[tk, tq] = k_rot @ q_rot^T per head; the
    two heads of a pair run CONCURRENTLY on the PE array (row-tiled, K=64
    each at row groups 0/64). softmax-exp is elementwise on ScalarE with
    scale=1/8 folded in; causal masking is a fixed 128x128 triangle on
    diagonal blocks (GpSimd); fully-masked blocks are skipped.
  - P@V also paired: M=64 per head (col groups 0/64) into one PSUM tile.
  - softmax denominators: a pair of 1-row ones-matmuls per block accumulates
    column sums in PSUM (also col-group paired); EA-matmul broadcasts them
    back over partitions; reciprocal_approx_fast + scale on VectorE.
  - The attention loop is software-pipelined: scores(kc+1) is emitted before
    PV(kc) so the PE never stalls behind the exp of the current block.
  - output projection per qb; PSUM evacuated by VectorE into a staging tile,
    one batched DMA per qb writes z.
"""

import math

import numpy as np
import ml_dtypes

import concourse.bass as bass
import concourse.bacc as bacc
import concourse.mybir as mybir
from concourse.tile import TileContext
from concourse.bass_utils import run_bass_kernel_spmd

BF16 = mybir.dt.bfloat16
F32 = mybir.dt.float32
NPBF16 = ml_dtypes.bfloat16

N_CORES = 8
P = 128


def build_attention_kernel(nc, T=2048, C=1024, n_heads=4, hd=64):
    """Emit the per-core kernel. Returns nothing; tensors are declared on nc."""
    HD = n_heads * hd            # 256: local head dims
    KC = C // P                  # 8: contraction chunks for projections
    NJC = HD // P                # 2: partition tiles of qT/kT (head pairs)
    TQB = 512                    # tq block for scores/PV
    NQB = T // TQB               # 4
    NTT = T // P                 # 16: t tiles for v
    scale = 1.0 / math.sqrt(hd)

    # ---- DRAM I/O ----
    xT = nc.declare_dram_parameter("xT", [C, T], BF16, isOutput=False)
    wqkv = nc.declare_dram_parameter("wqkv", [C, 3 * HD], BF16, isOutput=False)
    wpT = nc.declare_dram_parameter("wpT", [HD, C], BF16, isOutput=False)
    # f32 consts: bq(NJC) | bk(NJC) | bv(HD)
    NFC = 2 * NJC + HD
    fcst = nc.declare_dram_parameter("fcst", [P, NFC], F32, isOutput=False)
    # bf16 consts: j(P) | tri(P) | ea(P) | ones(2) | cos(T) | sin(T)
    NBC = 3 * P + 2 + 2 * T
    bcst = nc.declare_dram_parameter("bcst", [P, NBC], BF16, isOutput=False)
    z = nc.declare_dram_parameter("z", [T, C], F32, isOutput=True)

    with TileContext(nc) as tc:
        import contextlib

        with contextlib.ExitStack() as ctx:
            # ---- persistent SBUF pools ----
            pc = ctx.enter_context(tc.tile_pool(name="const", bufs=1))
            px = ctx.enter_context(tc.tile_pool(name="x", bufs=1))
            pw = ctx.enter_context(tc.tile_pool(name="w", bufs=1))
            pqk = ctx.enter_context(tc.tile_pool(name="qk", bufs=1))
            pv = ctx.enter_context(tc.tile_pool(name="v", bufs=1))
            py = ctx.enter_context(tc.tile_pool(name="y", bufs=1))
            # transient pools
            praw = ctx.enter_context(tc.tile_pool(name="raw", bufs=2))
            prt = ctx.enter_context(tc.tile_pool(name="ropetmp", bufs=4))
            pexp = ctx.enter_context(tc.tile_pool(name="exp", bufs=6))
            prcp = ctx.enter_context(tc.tile_pool(name="rcp", bufs=2))
            pzs = ctx.enter_context(tc.tile_pool(name="zstage", bufs=2))
            # PSUM pools (4 + 2 + 2 = 8 banks)
            psc = ctx.enter_context(
                tc.tile_pool(name="sc", bufs=2, space="PSUM"))
            pyt = ctx.enter_context(
                tc.tile_pool(name="yt", bufs=2, space="PSUM"))
            pms = ctx.enter_context(
                tc.tile_pool(name="ms", bufs=2, space="PSUM"))

            # ---- input DMAs, spread across engine queues ----
            # weights on the scalar queue
            t_wq = pw.tile([P, KC * HD], BF16, tag="wq")
            nc.scalar.dma_start(
                t_wq[:].rearrange("p (k n) -> p k n", k=KC),
                wqkv[:, 0:HD].rearrange("(k p) n -> p k n", k=KC),
            )
            # x chunks on the sync queue (chunk k feeds matmul k immediately)
            t_x = []
            for k in range(KC):
                x_t = px.tile([P, T], BF16, tag=f"x{k}")
                nc.sync.dma_start(x_t[:], xT[k * P:(k + 1) * P, :])
                t_x.append(x_t)
            t_wk = pw.tile([P, KC * HD], BF16, tag="wk")
            nc.scalar.dma_start(
                t_wk[:].rearrange("p (k n) -> p k n", k=KC),
                wqkv[:, HD:2 * HD].rearrange("(k p) n -> p k n", k=KC),
            )
            t_wv = pw.tile([P, KC * HD], BF16, tag="wv")
            nc.scalar.dma_start(
                t_wv[:].rearrange("p (k n) -> p k n", k=KC),
                wqkv[:, 2 * HD:3 * HD].rearrange("(k p) n -> p k n", k=KC),
            )
            t_fc = pc.tile([P, NFC], F32, tag="fc")
            nc.scalar.dma_start(t_fc[:], fcst[:])
            t_bc = pc.tile([P, NBC], BF16, tag="bc")
            nc.gpsimd.dma_start(t_bc[:], bcst[:])
            t_wp = pw.tile([P, NJC * C], BF16, tag="wp")
            nc.gpsimd.dma_start(
                t_wp[:].rearrange("p (j n) -> p j n", j=NJC),
                wpT[:].rearrange("(j p) n -> p j n", j=NJC),
            )

            # const views
            t_j = t_bc[:, 0:P]
            t_tri = t_bc[:, P:2 * P]
            t_ea = t_bc[:, 2 * P:3 * P]
            t_ones = t_bc[:, 3 * P:3 * P + 1]
            cos = t_bc[:, 3 * P + 2:3 * P + 2 + T]
            sin = t_bc[:, 3 * P + 2 + T:NBC]
            bq = t_fc[:, 0:NJC]
            bk = t_fc[:, NJC:2 * NJC]
            bv = t_fc[:, 2 * NJC:2 * NJC + HD]

            # persistent staging tile for softmax denominators (rows 0/64
            # carry data; the rest must be finite zeros for the EA matmul)
            t_scp = pc.tile([P, TQB], BF16, tag="scp")
            nc.vector.memset(t_scp[:], 0.0)

            # tri view broadcast over the 2-head dim
            def tri3():
                ap = t_tri
                return bass.AP(
                    ap.tensor, ap.offset,
                    [ap.ap[0], [0, 2], ap.ap[1]],
                )

            # ---- q/k projections (transposed) + RoPE ----
            t_qrot = [pqk.tile([P, T], BF16, tag=f"qr{jc}", name=f"qrot{jc}")
                      for jc in range(NJC)]
            t_krot = [pqk.tile([P, T], BF16, tag=f"kr{jc}", name=f"krot{jc}")
                      for jc in range(NJC)]

            for jc in range(NJC):
                for (wt, bias, dst) in (
                    (t_wq, bq, t_qrot[jc]),
                    (t_wk, bk, t_krot[jc]),
                ):
                    raw = praw.tile([P, T], BF16, tag="qkraw")
                    for tb in range(T // TQB):
                        qps = psc.tile([P, 2 * TQB], F32, tag="sc")
                        for k in range(KC):
                            nc.tensor.matmul(
                                qps[:, 0:TQB],
                                lhsT=wt[:, k * HD + jc * P:
                                        k * HD + (jc + 1) * P],
                                rhs=t_x[k][:, tb * TQB:(tb + 1) * TQB],
                                start=(k == 0),
                                stop=(k == KC - 1),
                            )
                        # evacuate + bias (bias cols are per-partition scalars)
                        nc.vector.tensor_scalar_add(
                            raw[:, tb * TQB:(tb + 1) * TQB],
                            qps[:, 0:TQB],
                            bias[:, jc:jc + 1],
                        )
                    # RoPE: rot = cos*raw + sin*(J@raw)
                    RW = 1024
                    for half in range(T // RW):
                        sl = slice(half * RW, (half + 1) * RW)
                        jps = psc.tile([P, RW], F32, tag="sc")
                        for qtr in range(RW // TQB):
                            nc.tensor.matmul(
                                jps[:, qtr * TQB:(qtr + 1) * TQB],
                                lhsT=t_j,
                                rhs=raw[:, sl][:, qtr * TQB:(qtr + 1) * TQB],
                            )
                        tmp1 = prt.tile([P, RW], BF16, tag="rope1")
                        nc.vector.tensor_mul(tmp1[:], raw[:, sl], cos[:, sl])
                        tmp2 = prt.tile([P, RW], BF16, tag="rope2")
                        with nc.allow_low_precision(reason="bf16 rope"):
                            nc.vector.tensor_mul(
                                tmp2[:], jps[:], sin[:, sl])
                            nc.vector.tensor_add(dst[:, sl], tmp1[:], tmp2[:])

            # ---- y_norm accumulators ----
            t_yn = [py.tile([P, T], BF16, tag=f"yn{jc}", name=f"yn{jc}")
                    for jc in range(NJC)]

            # ---- v projection emitter (chunks interleave with attention) ----
            t_v = [None] * NTT

            def emit_vproj(tt0, tt1):
                for tt in range(tt0, tt1):
                    vps = pyt.tile([P, TQB], F32, tag="yt")
                    for k in range(KC):
                        nc.tensor.matmul(
                            vps[:, 0:HD],
                            lhsT=t_x[k][:, tt * P:(tt + 1) * P],
                            rhs=t_wv[:, k * HD:(k + 1) * HD],
                            start=(k == 0),
                            stop=(k == KC - 1),
                        )
                    v_t = pv.tile([P, HD], BF16, tag=f"v{tt}")
                    nc.vector.tensor_add(v_t[:], vps[:, 0:HD], bv)
                    t_v[tt] = v_t

            # ---- attention (qb outer; vproj/zproj interleave) ----
            for qb in range(NQB):
                emit_vproj(qb * (TQB // P), (qb + 1) * (TQB // P))
                for hp in range(NJC):
                    n_kc = (qb + 1) * (TQB // P)
                    # software-pipelined: scores(kc+1) emitted before PV(kc)
                    scs = [None] * n_kc

                    def emit_scores(kc):
                        sc = psc.tile([P, 2 * TQB], F32, tag="sc")
                        for hl in range(2):
                            nc.tensor.matmul(
                                sc[:, hl * TQB:(hl + 1) * TQB],
                                lhsT=t_krot[hp][
                                    hl * hd:(hl + 1) * hd,
                                    kc * P:(kc + 1) * P],
                                rhs=t_qrot[hp][
                                    hl * hd:(hl + 1) * hd,
                                    qb * TQB:(qb + 1) * TQB],
                            )
                        scs[kc] = sc

                    yt = pyt.tile([P, TQB], F32, tag="yt")
                    dns = pms.tile([P, TQB], F32, tag="ms")
                    emit_scores(0)
                    for kc in range(n_kc):
                        sc = scs[kc]
                        # exp with 1/sqrt(hd) folded in; diag-trim left cols
                        s0 = max(0, kc * P - qb * TQB)
                        ex = pexp.tile([P, 2 * TQB], BF16, tag="exp")
                        sc3 = sc[:].rearrange("p (h w) -> p h w", h=2)
                        ex3 = ex[:].rearrange("p (h w) -> p h w", h=2)
                        if s0 > 0:
                            nc.gpsimd.memset(ex3[:, :, 0:s0], 0.0)
                        nc.scalar.activation(
                            ex3[:, :, s0:TQB],
                            sc3[:, :, s0:TQB],
                            mybir.ActivationFunctionType.Exp,
                            scale=scale,
                        )
                        if kc + 1 < n_kc:
                            emit_scores(kc + 1)
                        # diagonal 128-wide triangle mask (tk<=tq kept)
                        if kc * P >= qb * TQB:
                            nc.gpsimd.tensor_mul(
                                ex3[:, :, s0:s0 + P],
                                ex3[:, :, s0:s0 + P],
                                tri3(),
                            )
                        # P @ V: both heads concurrent (col groups 0/64)
                        for hl in range(2):
                            nc.tensor.matmul(
                                yt[hl * hd:(hl + 1) * hd, :],
                                lhsT=t_v[kc][
                                    :, (2 * hp + hl) * hd:
                                       (2 * hp + hl + 1) * hd],
                                rhs=ex[:, hl * TQB:(hl + 1) * TQB],
                                start=(kc == 0),
                                stop=(kc == n_kc - 1),
                                skip_group_check=True,
                            )
                        # denominators: paired 1-row ones-matmuls accumulate
                        # column sums over tk in PSUM rows 0/64
                        for hl in range(2):
                            nc.tensor.matmul(
                                dns[hl * hd:hl * hd + 1, :],
                                lhsT=t_ones,
                                rhs=ex[:, hl * TQB:(hl + 1) * TQB],
                                start=(kc == 0),
                                stop=(kc == n_kc - 1),
                                skip_group_check=True,
                            )
                    # stage rows, broadcast via EA matmul, reciprocal on DVE
                    with nc.allow_low_precision(reason="bf16 softmax denom"):
                        nc.vector.tensor_copy(
                            t_scp[0:1, :], dns[0:1, :])
                        nc.vector.tensor_copy(
                            t_scp[hd:hd + 1, :], dns[hd:hd + 1, :])
                    bc = pms.tile([P, TQB], F32, tag="ms")
                    nc.tensor.matmul(bc[:], lhsT=t_ea, rhs=t_scp[:])
                    rcpb = prcp.tile([P, TQB], F32, tag="rcpb")
                    nc.vector.reciprocal_approx_fast(rcpb[:], bc[:])
                    nc.vector.tensor_mul(
                        t_yn[hp][:, qb * TQB:(qb + 1) * TQB],
                        yt[:], rcpb[:])

                # ---- output projection for this qb ----
                zs = pzs.tile([P, (TQB // P) * C], F32, tag="zs")
                for m in range(TQB // P):
                    tt = qb * (TQB // P) + m
                    for co in range(C // TQB):
                        zps = pms.tile([P, TQB], F32, tag="ms")
                        for jc in range(NJC):
                            nc.tensor.matmul(
                                zps[:],
                                lhsT=t_yn[jc][:, tt * P:(tt + 1) * P],
                                rhs=t_wp[:, jc * C + co * TQB:
                                         jc * C + (co + 1) * TQB],
                                start=(jc == 0),
                                stop=(jc == NJC - 1),
                            )
                        nc.vector.tensor_copy(
                            zs[:, (m * (C // TQB) + co) * TQB:
                               (m * (C // TQB) + co + 1) * TQB],
                            zps[:])
                nc.gpsimd.dma_start(
                    z[qb * TQB:(qb + 1) * TQB, :].rearrange(
                        "(m p) (co c) -> p m co c", m=TQB // P, co=C // TQB),
                    zs[:].rearrange(
                        "p (m co c) -> p m co c", m=TQB // P, co=C // TQB),
                )


_ROPE_PERM = np.concatenate([np.arange(0, 64, 2), np.arange(1, 64, 2)])


def _host_inputs(x_b, Wq, bq, Wk, bk, Wv, bv, Wp, heads, T, C, hd):
    """Build the per-core DRAM input dict (numpy)."""
    HD = len(heads) * hd
    rows = np.concatenate([h * hd + _ROPE_PERM for h in heads])
    rows_nop = np.concatenate([np.arange(h * hd, (h + 1) * hd) for h in heads])

    xT = np.ascontiguousarray(x_b.T).astype(NPBF16)
    wqkv = np.concatenate(
        [Wq[rows].T, Wk[rows].T, Wv[rows_nop].T], axis=1).astype(NPBF16)
    wpT = np.ascontiguousarray(Wp[:, rows_nop].T).astype(NPBF16)

    j = np.arange(hd // 2, dtype=np.float64)
    inv_freq = 1.0 / (10000.0 ** (2.0 * j / hd))
    t = np.arange(T, dtype=np.float64)
    ang = t[:, None] * inv_freq[None, :]          # [T, 32]
    cosv = np.cos(ang)
    sinv = np.sin(ang)
    r = np.arange(P)
    cosq = cosv[:, r % (hd // 2)].T.astype(np.float32)
    sgn = np.where((r % hd) < hd // 2, -1.0, 1.0)
    sinsq = (sinv[:, r % (hd // 2)] * sgn[None, :]).T.astype(np.float32)

    pair = np.where((r % hd) < hd // 2, r + hd // 2, r - hd // 2)
    jmat = np.zeros((P, P), np.float32)
    jmat[pair, r] = 1.0
    tri = (np.arange(P)[None, :] >= np.arange(P)[:, None]).astype(np.float32)
    ea = np.zeros((P, P), np.float32)
    ea[(r // hd) * hd, r] = 1.0
    ones2 = np.ones((P, 2), np.float32)

    NJC = HD // P
    bqTh = bq[rows].reshape(NJC, P).T.astype(np.float32)
    bkTh = bk[rows].reshape(NJC, P).T.astype(np.float32)
    bvb = np.tile(bv[rows_nop][None, :], (P, 1)).astype(np.float32)

    fcst = np.concatenate([bqTh, bkTh, bvb], axis=1).astype(np.float32)
    bcst = np.concatenate(
        [jmat, tri, ea, ones2, cosq, sinsq], axis=1).astype(NPBF16)

    return {
        "xT": xT, "wqkv": wqkv, "wpT": wpT,
        "fcst": np.ascontiguousarray(fcst),
        "bcst": np.ascontiguousarray(bcst),
    }


def make_core_inputs(x, Wq, bq, Wk, bk, Wv, bv, Wp, T=2048, C=1024, hd=64,
                     heads_per_core=4):
    in_maps = []
    for c in range(N_CORES):
        b = c // 4
        g = c % 4
        heads = list(range(g * heads_per_core, (g + 1) * heads_per_core))
        in_maps.append(_host_inputs(
            np.asarray(x[b]), Wq, bq, Wk, bk, Wv, bv, Wp, heads, T, C, hd))
    return in_maps


def kernel(x, Wq, bq, Wk, bk, Wv, bv, Wp, bp):
    x = np.asarray(x, np.float32)
    Wq = np.asarray(Wq, np.float32)
    bq = np.asarray(bq, np.float32)
    Wk = np.asarray(Wk, np.float32)
    bk = np.asarray(bk, np.float32)
    Wv = np.asarray(Wv, np.float32)
    bv = np.asarray(bv, np.float32)
    Wp = np.asarray(Wp, np.float32)
    bp = np.asarray(bp, np.float32)
    B, T, C = x.shape

    nc = bacc.Bacc("TRN2", target_bir_lowering=False, debug=False,
                   num_devices=N_CORES)
    build_attention_kernel(nc, T=T, C=C)
    nc.compile()

    in_maps = make_core_inputs(x, Wq, bq, Wk, bk, Wv, bv, Wp, T=T, C=C)
    res = run_bass_kernel_spmd(nc, in_maps, list(range(N_CORES)))

    out = np.zeros((B, T, C), np.float32)
    for c in range(N_CORES):
        out[c // 4] += res.results[c]["z"]
    out += bp[None, None, :]
    return out


if __name__ == "__main__":
    import reference

    inputs = reference.setup_inputs()
    expected = np.asarray(reference.reference(**inputs))
    actual = kernel(**{k: np.asarray(v) for k, v in inputs.items()})
    err = np.abs(actual - expected).max() / np.abs(expected).max()
    print("Relative error:", err)


# revision 9
# speedup vs baseline: 1.3297x; 1.0091x over previous
"""Causal self-attention (B=2, T=2048, C=1024, H=16, RoPE) on 8 TRN2 cores.

Sharding: data-parallel over B (2 groups of 4 cores) x tensor-parallel over
heads (4 heads per core). Each core computes q/k/v projections for its heads,
RoPE, causal attention, and its partial output projection; the host sums the
4 partial projections per batch and adds bp.

Layout choices (per core):
  - xT [C, T] resident in SBUF (contraction dim C on partitions), loaded as 8
    per-chunk DMAs so the first projection matmul starts as soon as chunk 0
    lands; weights are host-pre-shuffled into [128, ...] layouts so every
    weight DMA is a cheap 2D transfer.
  - q, k produced TRANSPOSED: qT/kT [256=4heads*64, T] via lhsT=W, rhs=xT.
    Head-dim pairs are pre-permuted (evens|odds) in the weights so RoPE
    needs no strided access; the pair-swap is a constant permutation
    matmul (J), combine on VectorE with bf16 cos/sin.
  - v produced NON-transposed: [T, 256] via lhsT=xT, rhs=WvT.
  - scores computed transposed: ST[tk, tq] = k_rot @ q_rot^T per head; the
    two heads of a pair run CONCURRENTLY on the PE array (row-tiled, K=64
    each at row groups 0/64). The causal triangle on diagonal blocks is
    applied IN PSUM by accumulating a constant NEG upper-triangle matmul
    (ntri @ eye) on top of the scores, so softmax-exp (ScalarE, scale=1/8
    folded in) produces exact zeros there and nothing sits between exp and
    PV on the critical path. Fully-masked left columns are memset on GpSimd
    (hidden behind exp). Fully-masked blocks are skipped.
  - P@V also paired: M=64 per head (col groups 0/64) into one PSUM tile.
  - softmax denominators: a pair of 1-row ones-matmuls per block accumulates
    column sums in PSUM (also col-group paired); EA-matmul broadcasts them
    back over partitions; reciprocal_approx_fast + scale on VectorE.
  - The attention loop is software-pipelined: scores(kc+1) is emitted before
    PV(kc), and v-projection / output-projection matmuls are drip-fed
    between attention iterations so the PE never idles and ScalarE never
    starves at qb boundaries.
  - output projection per qb; PSUM evacuated by VectorE into a staging tile,
    one batched DMA per qb writes z.
"""

import math

import numpy as np
import ml_dtypes

import concourse.bass as bass
import concourse.bacc as bacc
import concourse.mybir as mybir
from concourse.tile import TileContext
from concourse.bass_utils import run_bass_kernel_spmd

BF16 = mybir.dt.bfloat16
F32 = mybir.dt.float32
NPBF16 = ml_dtypes.bfloat16

N_CORES = 8
P = 128
NEG = -1e9


def build_attention_kernel(nc, T=2048, C=1024, n_heads=4, hd=64):
    """Emit the per-core kernel. Returns nothing; tensors are declared on nc."""
    HD = n_heads * hd            # 256: local head dims
    KC = C // P                  # 8: contraction chunks for projections
    NJC = HD // P                # 2: partition tiles of qT/kT (head pairs)
    TQB = 512                    # tq block for scores/PV
    NQB = T // TQB               # 4
    NTT = T // P                 # 16: t tiles for v
    scale = 1.0 / math.sqrt(hd)

    # ---- DRAM I/O (weights host-pre-shuffled to [128, .] layouts) ----
    xT = nc.declare_dram_parameter("xT", [C, T], BF16, isOutput=False)
    wqs = nc.declare_dram_parameter("wqs", [P, KC * HD], BF16, isOutput=False)
    wks = nc.declare_dram_parameter("wks", [P, KC * HD], BF16, isOutput=False)
    wvs = nc.declare_dram_parameter("wvs", [P, KC * HD], BF16, isOutput=False)
    wps = nc.declare_dram_parameter("wps", [P, NJC * C], BF16, isOutput=False)
    # f32 consts: bq(NJC) | bk(NJC) | bv(HD)
    NFC = 2 * NJC + HD
    fcst = nc.declare_dram_parameter("fcst", [P, NFC], F32, isOutput=False)
    # bf16 consts: j | ea | ntri | eye | ones(2) | cos(T) | sin(T)
    NBC = 4 * P + 2 + 2 * T
    bcst = nc.declare_dram_parameter("bcst", [P, NBC], BF16, isOutput=False)
    z = nc.declare_dram_parameter("z", [T, C], F32, isOutput=True)

    with TileContext(nc) as tc:
        import contextlib

        with contextlib.ExitStack() as ctx:
            # ---- persistent SBUF pools ----
            pc = ctx.enter_context(tc.tile_pool(name="const", bufs=1))
            px = ctx.enter_context(tc.tile_pool(name="x", bufs=1))
            pw = ctx.enter_context(tc.tile_pool(name="w", bufs=1))
            pqk = ctx.enter_context(tc.tile_pool(name="qk", bufs=1))
            pv = ctx.enter_context(tc.tile_pool(name="v", bufs=1))
            py = ctx.enter_context(tc.tile_pool(name="y", bufs=1))
            # transient pools
            praw = ctx.enter_context(tc.tile_pool(name="raw", bufs=2))
            prt = ctx.enter_context(tc.tile_pool(name="ropetmp", bufs=4))
            pexp = ctx.enter_context(tc.tile_pool(name="exp", bufs=6))
            prcp = ctx.enter_context(tc.tile_pool(name="rcp", bufs=2))
            pzs = ctx.enter_context(tc.tile_pool(name="zstage", bufs=2))
            # PSUM pools (4 + 1 + 1 + 2 = 8 banks)
            psc = ctx.enter_context(
                tc.tile_pool(name="sc", bufs=2, space="PSUM"))
            pyt = ctx.enter_context(
                tc.tile_pool(name="yt", bufs=1, space="PSUM"))
            pms = ctx.enter_context(
                tc.tile_pool(name="ms", bufs=1, space="PSUM"))
            pz = ctx.enter_context(
                tc.tile_pool(name="zp", bufs=2, space="PSUM"))

            # ---- input DMAs, spread across engine queues ----
            t_wq = pw.tile([P, KC * HD], BF16, tag="wq")
            nc.scalar.dma_start(t_wq[:], wqs[:])
            # x chunks on the sync queue (chunk k feeds matmul k immediately)
            t_x = []
            for k in range(KC):
                x_t = px.tile([P, T], BF16, tag=f"x{k}")
                nc.sync.dma_start(x_t[:], xT[k * P:(k + 1) * P, :])
                t_x.append(x_t)
            t_wk = pw.tile([P, KC * HD], BF16, tag="wk")
            nc.scalar.dma_start(t_wk[:], wks[:])
            t_wv = pw.tile([P, KC * HD], BF16, tag="wv")
            nc.scalar.dma_start(t_wv[:], wvs[:])
            t_fc = pc.tile([P, NFC], F32, tag="fc")
            nc.scalar.dma_start(t_fc[:], fcst[:])
            t_bc = pc.tile([P, NBC], BF16, tag="bc")
            nc.gpsimd.dma_start(t_bc[:], bcst[:])
            t_wp = pw.tile([P, NJC * C], BF16, tag="wp")
            nc.gpsimd.dma_start(t_wp[:], wps[:])

            # const views
            t_j = t_bc[:, 0:P]
            t_ea = t_bc[:, P:2 * P]
            t_ntri = t_bc[:, 2 * P:3 * P]
            t_eye = t_bc[:, 3 * P:4 * P]
            t_ones = t_bc[:, 4 * P:4 * P + 1]
            cos = t_bc[:, 4 * P + 2:4 * P + 2 + T]
            sin = t_bc[:, 4 * P + 2 + T:NBC]
            bq = t_fc[:, 0:NJC]
            bk = t_fc[:, NJC:2 * NJC]
            bv = t_fc[:, 2 * NJC:2 * NJC + HD]

            # persistent staging tile for softmax denominators (rows 0/64
            # carry data; the rest must be finite zeros for the EA matmul)
            t_scp = pc.tile([P, TQB], BF16, tag="scp")
            nc.vector.memset(t_scp[:], 0.0)

            # ---- q/k projections (transposed) + RoPE ----
            t_qrot = [pqk.tile([P, T], BF16, tag=f"qr{jc}", name=f"qrot{jc}")
                      for jc in range(NJC)]
            t_krot = [pqk.tile([P, T], BF16, tag=f"kr{jc}", name=f"krot{jc}")
                      for jc in range(NJC)]

            for jc in range(NJC):
                for (wt, bias, dst) in (
                    (t_wq, bq, t_qrot[jc]),
                    (t_wk, bk, t_krot[jc]),
                ):
                    raw = praw.tile([P, T], BF16, tag="qkraw")
                    for tb in range(T // TQB):
                        qps = psc.tile([P, 2 * TQB], F32, tag="sc")
                        for k in range(KC):
                            nc.tensor.matmul(
                                qps[:, 0:TQB],
                                lhsT=wt[:, k * HD + jc * P:
                                        k * HD + (jc + 1) * P],
                                rhs=t_x[k][:, tb * TQB:(tb + 1) * TQB],
                                start=(k == 0),
                                stop=(k == KC - 1),
                            )
                        # evacuate + bias (bias cols are per-partition scalars)
                        nc.vector.tensor_scalar_add(
                            raw[:, tb * TQB:(tb + 1) * TQB],
                            qps[:, 0:TQB],
                            bias[:, jc:jc + 1],
                        )
                    # RoPE: rot = cos*raw + sin*(J@raw)
                    RW = 1024
                    for half in range(T // RW):
                        sl = slice(half * RW, (half + 1) * RW)
                        jps = psc.tile([P, RW], F32, tag="sc")
                        for qtr in range(RW // TQB):
                            nc.tensor.matmul(
                                jps[:, qtr * TQB:(qtr + 1) * TQB],
                                lhsT=t_j,
                                rhs=raw[:, sl][:, qtr * TQB:(qtr + 1) * TQB],
                            )
                        tmp1 = prt.tile([P, RW], BF16, tag="rope1")
                        nc.vector.tensor_mul(tmp1[:], raw[:, sl], cos[:, sl])
                        tmp2 = prt.tile([P, RW], BF16, tag="rope2")
                        with nc.allow_low_precision(reason="bf16 rope"):
                            nc.vector.tensor_mul(
                                tmp2[:], jps[:], sin[:, sl])
                            nc.vector.tensor_add(dst[:, sl], tmp1[:], tmp2[:])

            # ---- y_norm accumulators ----
            t_yn = [py.tile([P, T], BF16, tag=f"yn{jc}", name=f"yn{jc}")
                    for jc in range(NJC)]

            # ---- deferred-work emitters (drip-fed between attention kcs) ---
            t_v = [None] * NTT

            def vproj_thunk(tt):
                def emit():
                    vps = pz.tile([P, TQB], F32, tag="zp")
                    for k in range(KC):
                        nc.tensor.matmul(
                            vps[:, 0:HD],
                            lhsT=t_x[k][:, tt * P:(tt + 1) * P],
                            rhs=t_wv[:, k * HD:(k + 1) * HD],
                            start=(k == 0),
                            stop=(k == KC - 1),
                        )
                    v_t = pv.tile([P, HD], BF16, tag=f"v{tt}")
                    nc.vector.tensor_add(v_t[:], vps[:, 0:HD], bv)
                    t_v[tt] = v_t
                return emit

            def zproj_thunks(qb):
                """Output projection for query block qb, as 9 thunks."""
                zs = pzs.tile([P, (TQB // P) * C], F32, tag="zs")
                thunks = []

                def group(m, co):
                    def emit():
                        tt = qb * (TQB // P) + m
                        zps = pz.tile([P, TQB], F32, tag="zp")
                        for jc in range(NJC):
                            nc.tensor.matmul(
                                zps[:],
                                lhsT=t_yn[jc][:, tt * P:(tt + 1) * P],
                                rhs=t_wp[:, jc * C + co * TQB:
                                         jc * C + (co + 1) * TQB],
                                start=(jc == 0),
                                stop=(jc == NJC - 1),
                            )
                        nc.vector.tensor_copy(
                            zs[:, (m * (C // TQB) + co) * TQB:
                               (m * (C // TQB) + co + 1) * TQB],
                            zps[:])
                    return emit

                for m in range(TQB // P):
                    for co in range(C // TQB):
                        thunks.append(group(m, co))

                def dma():
                    nc.gpsimd.dma_start(
                        z[qb * TQB:(qb + 1) * TQB, :].rearrange(
                            "(m p) (co c) -> p m co c",
                            m=TQB // P, co=C // TQB),
                        zs[:].rearrange(
                            "p (m co c) -> p m co c",
                            m=TQB // P, co=C // TQB),
                    )
                thunks.append(dma)
                return thunks

            pending = [vproj_thunk(tt) for tt in range(TQB // P)]

            # ---- attention (qb outer; deferred work drip-fed between kcs) --
            for qb in range(NQB):
                # everything queued so far must land before this qb needs it
                while pending:
                    pending.pop(0)()
                for hp in range(NJC):
                    if hp == 0 and qb > 0:
                        pending.extend(zproj_thunks(qb - 1))
                    if hp == 1 and qb < NQB - 1:
                        pending.extend(
                            vproj_thunk(tt) for tt in
                            range((qb + 1) * (TQB // P), (qb + 2) * (TQB // P)))
                    n_kc = (qb + 1) * (TQB // P)
                    # software-pipelined: scores(kc+1) emitted before PV(kc)
                    scs = [None] * n_kc

                    def emit_scores(kc):
                        sc = psc.tile([P, 2 * TQB], F32, tag="sc")
                        sc3 = sc[:].rearrange("p (h w) -> p h w", h=2)
                        for hl in range(2):
                            nc.tensor.matmul(
                                sc[:, hl * TQB:(hl + 1) * TQB],
                                lhsT=t_krot[hp][
                                    hl * hd:(hl + 1) * hd,
                                    kc * P:(kc + 1) * P],
                                rhs=t_qrot[hp][
                                    hl * hd:(hl + 1) * hd,
                                    qb * TQB:(qb + 1) * TQB],
                            )
                        if kc * P >= qb * TQB:
                            # causal triangle: accumulate NEG upper-triangle
                            # onto the diagonal 128x128 block in PSUM
                            s0 = kc * P - qb * TQB
                            for hl in range(2):
                                nc.tensor.matmul(
                                    sc3[:, hl, s0:s0 + P],
                                    lhsT=t_ntri,
                                    rhs=t_eye,
                                    start=False,
                                    stop=True,
                                    skip_group_check=True,
                                )
                        scs[kc] = sc

                    yt = pyt.tile([P, TQB], F32, tag="yt")
                    dns = pms.tile([P, TQB], F32, tag="ms")
                    emit_scores(0)
                    for kc in range(n_kc):
                        sc = scs[kc]
                        # exp with 1/sqrt(hd) folded in; left cols fully
                        # masked for diagonal-group blocks
                        s0 = max(0, kc * P - qb * TQB)
                        ex = pexp.tile([P, 2 * TQB], BF16, tag="exp")
                        sc3 = sc[:].rearrange("p (h w) -> p h w", h=2)
                        ex3 = ex[:].rearrange("p (h w) -> p h w", h=2)
                        if s0 > 0:
                            nc.gpsimd.memset(ex3[:, :, 0:s0], 0.0)
                        nc.scalar.activation(
                            ex3[:, :, s0:TQB],
                            sc3[:, :, s0:TQB],
                            mybir.ActivationFunctionType.Exp,
                            scale=scale,
                        )
                        if kc + 1 < n_kc:
                            emit_scores(kc + 1)
                        elif pending:
                            pending.pop(0)()
                        # P @ V: both heads concurrent (col groups 0/64)
                        for hl in range(2):
                            nc.tensor.matmul(
                                yt[hl * hd:(hl + 1) * hd, :],
                                lhsT=t_v[kc][
                                    :, (2 * hp + hl) * hd:
                                       (2 * hp + hl + 1) * hd],
                                rhs=ex[:, hl * TQB:(hl + 1) * TQB],
                                start=(kc == 0),
                                stop=(kc == n_kc - 1),
                                skip_group_check=True,
                            )
                        # denominators: paired 1-row ones-matmuls accumulate
                        # column sums over tk in PSUM rows 0/64
                        for hl in range(2):
                            nc.tensor.matmul(
                                dns[hl * hd:hl * hd + 1, :],
                                lhsT=t_ones,
                                rhs=ex[:, hl * TQB:(hl + 1) * TQB],
                                start=(kc == 0),
                                stop=(kc == n_kc - 1),
                                skip_group_check=True,
                            )
                        # drip-feed one deferred vproj/zproj unit per kc
                        if pending:
                            pending.pop(0)()
                    # stage rows, broadcast via EA matmul, reciprocal on DVE
                    with nc.allow_low_precision(reason="bf16 softmax denom"):
                        nc.vector.tensor_copy(
                            t_scp[0:1, :], dns[0:1, :])
                        nc.vector.tensor_copy(
                            t_scp[hd:hd + 1, :], dns[hd:hd + 1, :])
                    bc = psc.tile([P, 2 * TQB], F32, tag="sc")
                    nc.tensor.matmul(bc[:, 0:TQB], lhsT=t_ea, rhs=t_scp[:])
                    rcpb = prcp.tile([P, TQB], F32, tag="rcpb")
                    nc.vector.reciprocal_approx_fast(rcpb[:], bc[:, 0:TQB])
                    nc.vector.tensor_mul(
                        t_yn[hp][:, qb * TQB:(qb + 1) * TQB],
                        yt[:], rcpb[:])

            for th in zproj_thunks(NQB - 1):
                th()


_ROPE_PERM = np.concatenate([np.arange(0, 64, 2), np.arange(1, 64, 2)])


def _shuf(w):
    """[C, N] -> [128, (C//128)*N]: chunk k of 128 rows -> cols [k*N,(k+1)*N)."""
    C, N = w.shape
    return np.ascontiguousarray(
        w.reshape(C // 128, 128, N).transpose(1, 0, 2).reshape(128, -1))


def _host_inputs(x_b, Wq, bq, Wk, bk, Wv, bv, Wp, heads, T, C, hd):
    """Build the per-core DRAM input dict (numpy)."""
    HD = len(heads) * hd
    rows = np.concatenate([h * hd + _ROPE_PERM for h in heads])
    rows_nop = np.concatenate([np.arange(h * hd, (h + 1) * hd) for h in heads])

    xT = np.ascontiguousarray(x_b.T).astype(NPBF16)
    wqs = _shuf(Wq[rows].T).astype(NPBF16)
    wks = _shuf(Wk[rows].T).astype(NPBF16)
    wvs = _shuf(Wv[rows_nop].T).astype(NPBF16)
    wps = _shuf(np.ascontiguousarray(Wp[:, rows_nop].T)).astype(NPBF16)

    j = np.arange(hd // 2, dtype=np.float64)
    inv_freq = 1.0 / (10000.0 ** (2.0 * j / hd))
    t = np.arange(T, dtype=np.float64)
    ang = t[:, None] * inv_freq[None, :]          # [T, 32]
    cosv = np.cos(ang)
    sinv = np.sin(ang)
    r = np.arange(P)
    cosq = cosv[:, r % (hd // 2)].T.astype(np.float32)
    sgn = np.where((r % hd) < hd // 2, -1.0, 1.0)
    sinsq = (sinv[:, r % (hd // 2)] * sgn[None, :]).T.astype(np.float32)

    pair = np.where((r % hd) < hd // 2, r + hd // 2, r - hd // 2)
    jmat = np.zeros((P, P), np.float32)
    jmat[pair, r] = 1.0
    ea = np.zeros((P, P), np.float32)
    ea[(r // hd) * hd, r] = 1.0
    # ntri.T @ eye adds NEG strictly above the diagonal (tk > tq)
    ntri = np.where(r[None, :] > r[:, None], NEG, 0.0).astype(np.float32)
    eye = np.eye(P, dtype=np.float32)
    ones2 = np.ones((P, 2), np.float32)

    NJC = HD // P
    bqTh = bq[rows].reshape(NJC, P).T.astype(np.float32)
    bkTh = bk[rows].reshape(NJC, P).T.astype(np.float32)
    bvb = np.tile(bv[rows_nop][None, :], (P, 1)).astype(np.float32)

    fcst = np.concatenate([bqTh, bkTh, bvb], axis=1).astype(np.float32)
    bcst = np.concatenate(
        [jmat, ea, ntri, eye, ones2, cosq, sinsq], axis=1).astype(NPBF16)

    return {
        "xT": xT, "wqs": wqs, "wks": wks, "wvs": wvs, "wps": wps,
        "fcst": np.ascontiguousarray(fcst),
        "bcst": np.ascontiguousarray(bcst),
    }


def make_core_inputs(x, Wq, bq, Wk, bk, Wv, bv, Wp, T=2048, C=1024, hd=64,
                     heads_per_core=4):
    in_maps = []
    for c in range(N_CORES):
        b = c // 4
        g = c % 4
        heads = list(range(g * heads_per_core, (g + 1) * heads_per_core))
        in_maps.append(_host_inputs(
            np.asarray(x[b]), Wq, bq, Wk, bk, Wv, bv, Wp, heads, T, C, hd))
    return in_maps


def kernel(x, Wq, bq, Wk, bk, Wv, bv, Wp, bp):
    x = np.asarray(x, np.float32)
    Wq = np.asarray(Wq, np.float32)
    bq = np.asarray(bq, np.float32)
    Wk = np.asarray(Wk, np.float32)
    bk = np.asarray(bk, np.float32)
    Wv = np.asarray(Wv, np.float32)
    bv = np.asarray(bv, np.float32)
    Wp = np.asarray(Wp, np.float32)
    bp = np.asarray(bp, np.float32)
    B, T, C = x.shape

    nc = bacc.Bacc("TRN2", target_bir_lowering=False, debug=False,
                   num_devices=N_CORES)
    build_attention_kernel(nc, T=T, C=C)
    nc.compile()

    in_maps = make_core_inputs(x, Wq, bq, Wk, bk, Wv, bv, Wp, T=T, C=C)
    res = run_bass_kernel_spmd(nc, in_maps, list(range(N_CORES)))

    out = np.zeros((B, T, C), np.float32)
    for c in range(N_CORES):
        out[c // 4] += res.results[c]["z"]
    out += bp[None, None, :]
    return out


if __name__ == "__main__":
    import reference

    inputs = reference.setup_inputs()
    expected = np.asarray(reference.reference(**inputs))
    actual = kernel(**{k: np.asarray(v) for k, v in inputs.items()})
    err = np.abs(actual - expected).max() / np.abs(expected).max()
    print("Relative error:", err)


# revision 12
# speedup vs baseline: 1.4328x; 1.0775x over previous
"""Causal self-attention (B=2, T=2048, C=1024, H=16, RoPE) on 8 TRN2 cores.

Sharding: data-parallel over B (2 groups of 4 cores) x tensor-parallel over
heads (4 heads per core). Each core computes q/k/v projections for its heads,
RoPE, causal attention, and its partial output projection; the host sums the
4 partial projections per batch and adds bp.

Layout choices (per core):
  - xT [C, T] resident in SBUF (contraction dim C on partitions), loaded as 8
    per-chunk DMAs so the first projection matmul starts as soon as chunk 0
    lands; weights are host-pre-shuffled into [128, ...] layouts so every
    weight DMA is a cheap 2D transfer.
  - q, k produced TRANSPOSED: qT/kT [256=4heads*64, T] via lhsT=W, rhs=xT.
    Head-dim pairs are pre-permuted (evens|odds) in the weights so RoPE
    needs no strided access; the pair-swap is a constant permutation
    matmul (J), combine on VectorE with bf16 cos/sin.
  - v produced NON-transposed: [T, 256] via lhsT=xT, rhs=WvT.
  - scores computed transposed: ST[tk, tq] = k_rot @ q_rot^T per head; the
    two heads of a pair run CONCURRENTLY on the PE array (row-tiled, K=64
    each at row groups 0/64). The causal triangle on diagonal blocks is
    applied IN PSUM by accumulating a constant NEG upper-triangle matmul
    (ntri @ eye) on top of the scores, so softmax-exp (ScalarE, scale=1/8
    folded in) produces exact zeros there and nothing sits between exp and
    PV on the critical path. Fully-masked left columns are memset on GpSimd
    (hidden behind exp). Fully-masked blocks are skipped.
  - P@V also paired: M=64 per head (col groups 0/64) into one PSUM tile.
  - softmax denominators: a pair of 1-row ones-matmuls per block accumulates
    column sums in PSUM (also col-group paired); EA-matmul broadcasts them
    back over partitions; reciprocal_approx_fast + scale on VectorE.
  - The attention loop is software-pipelined: scores(kc+1) is emitted before
    PV(kc), and v-projection / output-projection matmuls are drip-fed
    between attention iterations so the PE never idles and ScalarE never
    starves at qb boundaries.
  - output projection per qb; PSUM evacuated by VectorE into a staging tile,
    one batched DMA per qb writes z.
"""

import math

import numpy as np
import ml_dtypes

import concourse.bass as bass
import concourse.bacc as bacc
import concourse.mybir as mybir
from concourse.tile import TileContext
from concourse.bass_utils import run_bass_kernel_spmd

BF16 = mybir.dt.bfloat16
F32 = mybir.dt.float32
NPBF16 = ml_dtypes.bfloat16

N_CORES = 8
P = 128
NEG = -1e9


def build_attention_kernel(nc, T=2048, C=1024, n_heads=4, hd=64):
    """Emit the per-core kernel. Returns nothing; tensors are declared on nc."""
    HD = n_heads * hd            # 256: local head dims
    KC = C // P                  # 8: contraction chunks for projections
    NJC = HD // P                # 2: partition tiles of qT/kT (head pairs)
    TQB = 512                    # tq block for scores/PV
    NQB = T // TQB               # 4
    NTT = T // P                 # 16: t tiles for v
    scale = 1.0 / math.sqrt(hd)

    # ---- DRAM I/O (weights host-pre-shuffled to [128, .] layouts) ----
    xT = nc.declare_dram_parameter("xT", [C, T], BF16, isOutput=False)
    wqs = nc.declare_dram_parameter("wqs", [P, KC * HD], BF16, isOutput=False)
    wks = nc.declare_dram_parameter("wks", [P, KC * HD], BF16, isOutput=False)
    wvs = nc.declare_dram_parameter("wvs", [P, KC * HD], BF16, isOutput=False)
    wps = nc.declare_dram_parameter("wps", [P, NJC * C], BF16, isOutput=False)
    # f32 consts: bq(NJC) | bk(NJC) | bv(HD)
    NFC = 2 * NJC + HD
    fcst = nc.declare_dram_parameter("fcst", [P, NFC], F32, isOutput=False)
    # bf16 consts: j | ea | ntri | eye | ones(2) | cos(T) | sin(T)
    NBC = 4 * P + 2 + 2 * T
    bcst = nc.declare_dram_parameter("bcst", [P, NBC], BF16, isOutput=False)
    z = nc.declare_dram_parameter("z", [T, C], F32, isOutput=True)

    with TileContext(nc) as tc:
        import contextlib

        with contextlib.ExitStack() as ctx:
            # ---- persistent SBUF pools ----
            pc = ctx.enter_context(tc.tile_pool(name="const", bufs=1))
            px = ctx.enter_context(tc.tile_pool(name="x", bufs=1))
            pw = ctx.enter_context(tc.tile_pool(name="w", bufs=1))
            pqk = ctx.enter_context(tc.tile_pool(name="qk", bufs=1))
            pv = ctx.enter_context(tc.tile_pool(name="v", bufs=1))
            py = ctx.enter_context(tc.tile_pool(name="y", bufs=1))
            # transient pools
            praw = ctx.enter_context(tc.tile_pool(name="raw", bufs=2))
            prt = ctx.enter_context(tc.tile_pool(name="ropetmp", bufs=4))
            pexp = ctx.enter_context(tc.tile_pool(name="exp", bufs=6))
            prcp = ctx.enter_context(tc.tile_pool(name="rcp", bufs=2))
            pzs = ctx.enter_context(tc.tile_pool(name="zstage", bufs=2))
            # PSUM pools (4 + 1 + 1 + 2 = 8 banks)
            psc = ctx.enter_context(
                tc.tile_pool(name="sc", bufs=2, space="PSUM"))
            pyt = ctx.enter_context(
                tc.tile_pool(name="yt", bufs=1, space="PSUM"))
            pms = ctx.enter_context(
                tc.tile_pool(name="ms", bufs=1, space="PSUM"))
            pz = ctx.enter_context(
                tc.tile_pool(name="zp", bufs=2, space="PSUM"))

            # ---- input DMAs, spread across engine queues ----
            t_wq = pw.tile([P, KC * HD], BF16, tag="wq")
            nc.scalar.dma_start(t_wq[:], wqs[:])
            # x chunks on the sync queue (chunk k feeds matmul k immediately)
            t_x = []
            for k in range(KC):
                x_t = px.tile([P, T], BF16, tag=f"x{k}")
                nc.sync.dma_start(x_t[:], xT[k * P:(k + 1) * P, :])
                t_x.append(x_t)
            t_wk = pw.tile([P, KC * HD], BF16, tag="wk")
            nc.scalar.dma_start(t_wk[:], wks[:])
            t_wv = pw.tile([P, KC * HD], BF16, tag="wv")
            nc.scalar.dma_start(t_wv[:], wvs[:])
            t_fc = pc.tile([P, NFC], F32, tag="fc")
            nc.scalar.dma_start(t_fc[:], fcst[:])
            t_bc = pc.tile([P, NBC], BF16, tag="bc")
            nc.gpsimd.dma_start(t_bc[:], bcst[:])
            t_wp = pw.tile([P, NJC * C], BF16, tag="wp")
            nc.gpsimd.dma_start(t_wp[:], wps[:])

            # const views
            t_j = t_bc[:, 0:P]
            t_ea = t_bc[:, P:2 * P]
            t_ntri = t_bc[:, 2 * P:3 * P]
            t_eye = t_bc[:, 3 * P:4 * P]
            t_ones = t_bc[:, 4 * P:4 * P + 1]
            cos = t_bc[:, 4 * P + 2:4 * P + 2 + T]
            sin = t_bc[:, 4 * P + 2 + T:NBC]
            bq = t_fc[:, 0:NJC]
            bk = t_fc[:, NJC:2 * NJC]
            bv = t_fc[:, 2 * NJC:2 * NJC + HD]

            # persistent staging tile for softmax denominators (rows 0/64
            # carry data; the rest must be finite zeros for the EA matmul)
            t_scp = pc.tile([P, TQB], BF16, tag="scp")
            nc.vector.memset(t_scp[:], 0.0)

            # ---- q/k projections (transposed) + RoPE ----
            t_qrot = [pqk.tile([P, T], BF16, tag=f"qr{jc}", name=f"qrot{jc}")
                      for jc in range(NJC)]
            t_krot = [pqk.tile([P, T], BF16, tag=f"kr{jc}", name=f"krot{jc}")
                      for jc in range(NJC)]

            for jc in range(NJC):
                for (wt, bias, dst) in (
                    (t_wq, bq, t_qrot[jc]),
                    (t_wk, bk, t_krot[jc]),
                ):
                    raw = praw.tile([P, T], BF16, tag="qkraw")
                    for tb in range(T // TQB):
                        qps = psc.tile([P, 2 * TQB], F32, tag="sc")
                        for k in range(KC):
                            nc.tensor.matmul(
                                qps[:, 0:TQB],
                                lhsT=wt[:, k * HD + jc * P:
                                        k * HD + (jc + 1) * P],
                                rhs=t_x[k][:, tb * TQB:(tb + 1) * TQB],
                                start=(k == 0),
                                stop=(k == KC - 1),
                            )
                        # evacuate + bias (bias cols are per-partition scalars)
                        nc.vector.tensor_scalar_add(
                            raw[:, tb * TQB:(tb + 1) * TQB],
                            qps[:, 0:TQB],
                            bias[:, jc:jc + 1],
                        )
                    # RoPE: rot = cos*raw + sin*(J@raw)
                    RW = 1024
                    for half in range(T // RW):
                        sl = slice(half * RW, (half + 1) * RW)
                        jps = psc.tile([P, RW], F32, tag="sc")
                        for qtr in range(RW // TQB):
                            nc.tensor.matmul(
                                jps[:, qtr * TQB:(qtr + 1) * TQB],
                                lhsT=t_j,
                                rhs=raw[:, sl][:, qtr * TQB:(qtr + 1) * TQB],
                            )
                        tmp1 = prt.tile([P, RW], BF16, tag="rope1")
                        nc.vector.tensor_mul(tmp1[:], raw[:, sl], cos[:, sl])
                        tmp2 = prt.tile([P, RW], BF16, tag="rope2")
                        with nc.allow_low_precision(reason="bf16 rope"):
                            nc.vector.tensor_mul(
                                tmp2[:], jps[:], sin[:, sl])
                            nc.vector.tensor_add(dst[:, sl], tmp1[:], tmp2[:])

            # ---- y_norm accumulators ----
            t_yn = [py.tile([P, T], BF16, tag=f"yn{jc}", name=f"yn{jc}")
                    for jc in range(NJC)]

            # ---- deferred-work emitters (drip-fed between attention kcs) ---
            t_v = [None] * NTT

            def vproj_thunk(tt):
                def emit():
                    vps = pz.tile([P, TQB], F32, tag="zp")
                    for k in range(KC):
                        nc.tensor.matmul(
                            vps[:, 0:HD],
                            lhsT=t_x[k][:, tt * P:(tt + 1) * P],
                            rhs=t_wv[:, k * HD:(k + 1) * HD],
                            start=(k == 0),
                            stop=(k == KC - 1),
                        )
                    v_t = pv.tile([P, HD], BF16, tag=f"v{tt}")
                    nc.vector.tensor_add(v_t[:], vps[:, 0:HD], bv)
                    t_v[tt] = v_t
                return emit

            def zproj_thunks(qb):
                """Output projection for query block qb, as 9 thunks."""
                zs = pzs.tile([P, (TQB // P) * C], F32, tag="zs")
                thunks = []

                def group(m, co):
                    def emit():
                        tt = qb * (TQB // P) + m
                        zps = pz.tile([P, TQB], F32, tag="zp")
                        for jc in range(NJC):
                            nc.tensor.matmul(
                                zps[:],
                                lhsT=t_yn[jc][:, tt * P:(tt + 1) * P],
                                rhs=t_wp[:, jc * C + co * TQB:
                                         jc * C + (co + 1) * TQB],
                                start=(jc == 0),
                                stop=(jc == NJC - 1),
                            )
                        nc.vector.tensor_copy(
                            zs[:, (m * (C // TQB) + co) * TQB:
                               (m * (C // TQB) + co + 1) * TQB],
                            zps[:])
                    return emit

                for m in range(TQB // P):
                    for co in range(C // TQB):
                        thunks.append(group(m, co))

                def dma():
                    nc.gpsimd.dma_start(
                        z[qb * TQB:(qb + 1) * TQB, :].rearrange(
                            "(m p) (co c) -> p m co c",
                            m=TQB // P, co=C // TQB),
                        zs[:].rearrange(
                            "p (m co c) -> p m co c",
                            m=TQB // P, co=C // TQB),
                    )
                thunks.append(dma)
                return thunks

            pending = [vproj_thunk(tt) for tt in range(TQB // P)]

            # stride-0 head-broadcast view of eye for the NTRI accumulate
            eye2 = bass.AP(
                t_eye.tensor, t_eye.offset,
                [t_eye.ap[0], [0, 2], t_eye.ap[1]])

            def emit_scores(qb, hp, kc):
                """Scores pair (+ causal NEG triangle) for one tk block."""
                sc = psc.tile([P, 2 * TQB], F32, tag="sc")
                sc3 = sc[:].rearrange("p (h w) -> p h w", h=2)
                s0 = max(0, kc * P - qb * TQB)
                for hl in range(2):
                    nc.tensor.matmul(
                        sc3[:, hl, s0:TQB],
                        lhsT=t_krot[hp][
                            hl * hd:(hl + 1) * hd,
                            kc * P:(kc + 1) * P],
                        rhs=t_qrot[hp][
                            hl * hd:(hl + 1) * hd,
                            qb * TQB + s0:(qb + 1) * TQB],
                    )
                if kc * P >= qb * TQB:
                    # causal triangle: accumulate NEG upper-triangle onto
                    # the diagonal 128x128 block (both heads, one matmul)
                    nc.tensor.matmul(
                        sc3[:, :, s0:s0 + P],
                        lhsT=t_ntri,
                        rhs=eye2,
                        start=False,
                        stop=True,
                        skip_group_check=True,
                    )
                return sc

            # ---- attention groups, software-pipelined one group ahead ----
            groups = [(qb, hp) for qb in range(NQB) for hp in range(NJC)]
            carried_sc = None
            for gi, (qb, hp) in enumerate(groups):
                if hp == 0:
                    # everything queued must land before this qb needs it
                    while pending:
                        pending.pop(0)()
                    if qb > 0:
                        pending.extend(zproj_thunks(qb - 1))
                elif qb < NQB - 1:
                    pending.extend(
                        vproj_thunk(tt) for tt in
                        range((qb + 1) * (TQB // P), (qb + 2) * (TQB // P)))
                n_kc = (qb + 1) * (TQB // P)
                scs = [None] * n_kc
                scs[0] = carried_sc if carried_sc is not None \
                    else emit_scores(qb, hp, 0)
                carried_sc = None
                yt = pyt.tile([P, TQB], F32, tag="yt")
                dns = pms.tile([P, TQB], F32, tag="ms")
                for kc in range(n_kc):
                    sc = scs[kc]
                    # exp with 1/sqrt(hd) folded in; left cols fully
                    # masked for diagonal-group blocks
                    s0 = max(0, kc * P - qb * TQB)
                    ex = pexp.tile([P, 2 * TQB], BF16, tag="exp")
                    sc3 = sc[:].rearrange("p (h w) -> p h w", h=2)
                    ex3 = ex[:].rearrange("p (h w) -> p h w", h=2)
                    if s0 > 0:
                        nc.gpsimd.memset(ex3[:, :, 0:s0], 0.0)
                    nc.scalar.activation(
                        ex3[:, :, s0:TQB],
                        sc3[:, :, s0:TQB],
                        mybir.ActivationFunctionType.Exp,
                        scale=scale,
                    )
                    if kc + 1 < n_kc:
                        scs[kc + 1] = emit_scores(qb, hp, kc + 1)
                    elif gi + 1 < len(groups):
                        # prefetch the next group's first scores so exp
                        # there overlaps this group's epilogue
                        carried_sc = emit_scores(*groups[gi + 1], 0)
                    # P @ V: both heads concurrent (col groups 0/64);
                    # masked left cols skipped (zero contribution)
                    for hl in range(2):
                        nc.tensor.matmul(
                            yt[hl * hd:(hl + 1) * hd, s0:TQB],
                            lhsT=t_v[kc][
                                :, (2 * hp + hl) * hd:
                                   (2 * hp + hl + 1) * hd],
                            rhs=ex3[:, hl, s0:TQB],
                            start=(kc == 0),
                            stop=(kc == n_kc - 1),
                            skip_group_check=True,
                        )
                    # denominators: paired 1-row ones-matmuls accumulate
                    # column sums over tk in PSUM rows 0/64
                    for hl in range(2):
                        nc.tensor.matmul(
                            dns[hl * hd:hl * hd + 1, s0:TQB],
                            lhsT=t_ones,
                            rhs=ex3[:, hl, s0:TQB],
                            start=(kc == 0),
                            stop=(kc == n_kc - 1),
                            skip_group_check=True,
                        )
                    # drip-feed one deferred vproj/zproj unit per kc
                    if pending:
                        pending.pop(0)()
                # stage rows, EA broadcast, reciprocal + scale on DVE
                with nc.allow_low_precision(reason="bf16 softmax denom"):
                    nc.vector.tensor_copy(t_scp[0:1, :], dns[0:1, :])
                    nc.vector.tensor_copy(
                        t_scp[hd:hd + 1, :], dns[hd:hd + 1, :])
                bc = pz.tile([P, TQB], F32, tag="zp")
                nc.tensor.matmul(bc[:], lhsT=t_ea, rhs=t_scp[:])
                rcpb = prcp.tile([P, TQB], F32, tag="rcpb")
                nc.vector.reciprocal_approx_fast(rcpb[:], bc[:])
                nc.vector.tensor_mul(
                    t_yn[hp][:, qb * TQB:(qb + 1) * TQB],
                    yt[:], rcpb[:])

            for th in zproj_thunks(NQB - 1):
                th()


_ROPE_PERM = np.concatenate([np.arange(0, 64, 2), np.arange(1, 64, 2)])


def _shuf(w):
    """[C, N] -> [128, (C//128)*N]: chunk k of 128 rows -> cols [k*N,(k+1)*N)."""
    C, N = w.shape
    return np.ascontiguousarray(
        w.reshape(C // 128, 128, N).transpose(1, 0, 2).reshape(128, -1))


def _host_inputs(x_b, Wq, bq, Wk, bk, Wv, bv, Wp, heads, T, C, hd):
    """Build the per-core DRAM input dict (numpy)."""
    HD = len(heads) * hd
    rows = np.concatenate([h * hd + _ROPE_PERM for h in heads])
    rows_nop = np.concatenate([np.arange(h * hd, (h + 1) * hd) for h in heads])

    xT = np.ascontiguousarray(x_b.T).astype(NPBF16)
    wqs = _shuf(Wq[rows].T).astype(NPBF16)
    wks = _shuf(Wk[rows].T).astype(NPBF16)
    wvs = _shuf(Wv[rows_nop].T).astype(NPBF16)
    wps = _shuf(np.ascontiguousarray(Wp[:, rows_nop].T)).astype(NPBF16)

    j = np.arange(hd // 2, dtype=np.float64)
    inv_freq = 1.0 / (10000.0 ** (2.0 * j / hd))
    t = np.arange(T, dtype=np.float64)
    ang = t[:, None] * inv_freq[None, :]          # [T, 32]
    cosv = np.cos(ang)
    sinv = np.sin(ang)
    r = np.arange(P)
    cosq = cosv[:, r % (hd // 2)].T.astype(np.float32)
    sgn = np.where((r % hd) < hd // 2, -1.0, 1.0)
    sinsq = (sinv[:, r % (hd // 2)] * sgn[None, :]).T.astype(np.float32)

    pair = np.where((r % hd) < hd // 2, r + hd // 2, r - hd // 2)
    jmat = np.zeros((P, P), np.float32)
    jmat[pair, r] = 1.0
    ea = np.zeros((P, P), np.float32)
    ea[(r // hd) * hd, r] = 1.0
    # ntri.T @ eye adds NEG strictly above the diagonal (tk > tq)
    ntri = np.where(r[None, :] > r[:, None], NEG, 0.0).astype(np.float32)
    eye = np.eye(P, dtype=np.float32)
    ones2 = np.ones((P, 2), np.float32)

    NJC = HD // P
    bqTh = bq[rows].reshape(NJC, P).T.astype(np.float32)
    bkTh = bk[rows].reshape(NJC, P).T.astype(np.float32)
    bvb = np.tile(bv[rows_nop][None, :], (P, 1)).astype(np.float32)

    fcst = np.concatenate([bqTh, bkTh, bvb], axis=1).astype(np.float32)
    bcst = np.concatenate(
        [jmat, ea, ntri, eye, ones2, cosq, sinsq], axis=1).astype(NPBF16)

    return {
        "xT": xT, "wqs": wqs, "wks": wks, "wvs": wvs, "wps": wps,
        "fcst": np.ascontiguousarray(fcst),
        "bcst": np.ascontiguousarray(bcst),
    }


def make_core_inputs(x, Wq, bq, Wk, bk, Wv, bv, Wp, T=2048, C=1024, hd=64,
                     heads_per_core=4):
    in_maps = []
    for c in range(N_CORES):
        b = c // 4
        g = c % 4
        heads = list(range(g * heads_per_core, (g + 1) * heads_per_core))
        in_maps.append(_host_inputs(
            np.asarray(x[b]), Wq, bq, Wk, bk, Wv, bv, Wp, heads, T, C, hd))
    return in_maps


def kernel(x, Wq, bq, Wk, bk, Wv, bv, Wp, bp):
    x = np.asarray(x, np.float32)
    Wq = np.asarray(Wq, np.float32)
    bq = np.asarray(bq, np.float32)
    Wk = np.asarray(Wk, np.float32)
    bk = np.asarray(bk, np.float32)
    Wv = np.asarray(Wv, np.float32)
    bv = np.asarray(bv, np.float32)
    Wp = np.asarray(Wp, np.float32)
    bp = np.asarray(bp, np.float32)
    B, T, C = x.shape

    nc = bacc.Bacc("TRN2", target_bir_lowering=False, debug=False,
                   num_devices=N_CORES)
    build_attention_kernel(nc, T=T, C=C)
    nc.compile()

    in_maps = make_core_inputs(x, Wq, bq, Wk, bk, Wv, bv, Wp, T=T, C=C)
    res = run_bass_kernel_spmd(nc, in_maps, list(range(N_CORES)))

    out = np.zeros((B, T, C), np.float32)
    for c in range(N_CORES):
        out[c // 4] += res.results[c]["z"]
    out += bp[None, None, :]
    return out


if __name__ == "__main__":
    import reference

    inputs = reference.setup_inputs()
    expected = np.asarray(reference.reference(**inputs))
    actual = kernel(**{k: np.asarray(v) for k, v in inputs.items()})
    err = np.abs(actual - expected).max() / np.abs(expected).max()
    print("Relative error:", err)
